# revision 9
# baseline (speedup 1.0000x reference)
"""Trainium2 Bass kernel for nn_MergeMetaCNN (hypernetwork MLP -> grouped conv -> CNN).

Data-parallel over batch: 32 samples -> 8 NeuronCores, 4 samples each.

Per-core pipeline (all math on device):
  1. MLP (fp32 matmuls): hid = relu(W1^T fxT + b1); rawT = W2^T hid + b2,
     scaled per-row by 0.1/27 (filter part) / 0.1 (bias part).
  2. conv1 (grouped 3x3, per-sample dynamic filters) as ONE matmul pass:
     block-diagonal stationary [4*27, 4*8] (bf16), moving operand = im2col
     tile [108, rows, 226] DMA-gathered from zero-padded bf16 X planes
     (each partition = contiguous shifted window of a padded plane).
  3. conv2 (8->64, 3x3) per sample: stationary [72, 64], moving = im2col
     [72, rows, 226] gathered from padded bf16 y planes.
  4. Epilogue relu(x + b) split across ScalarE/VectorE, bf16 staging,
     upcast to fp32 during the store DMA (SWDGE cast).
"""

import numpy as np
from contextlib import ExitStack

import concourse.bass as bass
import concourse.tile as tile
from concourse import bacc, mybir
from concourse.bass_utils import run_bass_kernel_spmd

AP = bass.AP
f32 = mybir.dt.float32
bf16 = mybir.dt.bfloat16
AF = mybir.ActivationFunctionType
ALU = mybir.AluOpType

# Problem constants (hardcoded per contract)
B, CIN, H, W = 32, 3, 224, 224
TMP, K, FLAT, COUT = 8, 3, 128, 64
MLP_OUT = TMP * CIN * K * K + TMP  # 224
META = 0.1
NCORES = 8
SPC = B // NCORES                  # 4 samples per core
PH, PW = H + 2, W + 2              # 226 (zero-pad 1 on each side)
PLANE = PH * PW                    # 51076
PP = PLANE + 4                     # padded plane stride (tail slack for windows)
K27 = CIN * K * K                  # 27
K72 = TMP * K * K                  # 72
RT = 16                            # image rows per row-tile
NRT = H // RT                      # 14 row-tiles
RMM = 2                            # rows per matmul (PSUM bank: 448 fp32 <= 512)
NFREE = RMM * W                    # 448
MM_PER_TILE = RT // RMM            # 4

_CACHE = {}


def build_module():
    """Build + compile the single-core Bass module (SPMD across 8 cores)."""
    if "nc" in _CACHE:
        return _CACHE["nc"]
    nc = bacc.Bacc("TRN2", target_bir_lowering=False, debug=False, num_devices=NCORES)

    # ---- DRAM I/O (per-core shapes) ----
    padX = nc.dram_tensor("padX", [SPC * CIN, PP], f32, kind="ExternalInput")
    fxT = nc.dram_tensor("fxT", [FLAT, SPC], f32, kind="ExternalInput")
    W1 = nc.dram_tensor("W1", [FLAT, MLP_OUT], f32, kind="ExternalInput")
    b1 = nc.dram_tensor("b1", [MLP_OUT], f32, kind="ExternalInput")
    W2 = nc.dram_tensor("W2", [MLP_OUT, MLP_OUT], f32, kind="ExternalInput")
    b2 = nc.dram_tensor("b2", [MLP_OUT], f32, kind="ExternalInput")
    cnn_wT = nc.dram_tensor("cnn_wT", [K72, COUT], f32, kind="ExternalInput")
    cnn_b = nc.dram_tensor("cnn_b", [COUT], f32, kind="ExternalInput")
    out = nc.dram_tensor("out", [SPC, COUT, H * W], f32, kind="ExternalOutput")

    padXb = nc.dram_tensor("padXb", [SPC * CIN, PP], bf16)   # bf16 cast of padX
    rawT_d = nc.dram_tensor("rawT_d", [MLP_OUT, SPC], f32)   # MLP out scratch

    with tile.TileContext(nc) as tc, ExitStack() as ctx:
        cpool = ctx.enter_context(tc.tile_pool(name="consts", bufs=1))
        spool = ctx.enter_context(tc.tile_pool(name="stageA", bufs=1))
        mpsum = ctx.enter_context(tc.tile_pool(name="mlp_psum", bufs=2, space="PSUM"))

        # ================= Stage A: MLP + weight prep =================
        w1sb = cpool.tile([FLAT, MLP_OUT], f32)
        nc.sync.dma_start(w1sb[:], W1.ap())
        w2a = cpool.tile([128, MLP_OUT], f32)
        nc.sync.dma_start(w2a[:], W2.ap()[0:128, :])
        w2b = cpool.tile([96, MLP_OUT], f32)
        nc.sync.dma_start(w2b[:], W2.ap()[128:224, :])
        fx_sb = cpool.tile([FLAT, SPC], f32)
        nc.sync.dma_start(fx_sb[:], fxT.ap())
        b1a = cpool.tile([128, 1], f32)
        nc.sync.dma_start(b1a[:], b1.ap()[0:128].unsqueeze(1))
        b1b = cpool.tile([96, 1], f32)
        nc.sync.dma_start(b1b[:], b1.ap()[128:224].unsqueeze(1))
        b2a = cpool.tile([128, 1], f32)
        nc.sync.dma_start(b2a[:], b2.ap()[0:128].unsqueeze(1))
        b2b = cpool.tile([96, 1], f32)
        nc.sync.dma_start(b2b[:], b2.ap()[128:224].unsqueeze(1))
        cnnb_sb = cpool.tile([COUT, 1], f32)
        nc.sync.dma_start(cnnb_sb[:], cnn_b.ap().unsqueeze(1))
        lhsT2 = cpool.tile([K72, COUT], bf16)
        nc.gpsimd.dma_start(lhsT2[:], cnn_wT.ap())  # cast f32 -> bf16

        # uniform scale 0.1/27 on all raw rows; bias rows corrected by x27 later
        WSCALE = META / K27
        b2v_a = cpool.tile([128, 1], f32)
        nc.vector.tensor_scalar_mul(b2v_a[:], b2a[:], WSCALE)
        b2v_b = cpool.tile([96, 1], f32)
        nc.vector.tensor_scalar_mul(b2v_b[:], b2b[:], WSCALE)

        # hid^T = relu(W1^T @ fxT + b1)   [224, SPC] in two partition chunks
        ph_a = mpsum.tile([128, SPC], f32, tag="mp")
        nc.tensor.matmul(ph_a[:], lhsT=w1sb[:, 0:128], rhs=fx_sb[:], start=True, stop=True)
        hida = spool.tile([128, SPC], f32)
        nc.scalar.activation(hida[:], ph_a[:], func=AF.Relu, bias=b1a[:])
        ph_b = mpsum.tile([96, SPC], f32, tag="mp")
        nc.tensor.matmul(ph_b[:], lhsT=w1sb[:, 128:224], rhs=fx_sb[:], start=True, stop=True)
        hidb = spool.tile([96, SPC], f32)
        nc.scalar.activation(hidb[:], ph_b[:], func=AF.Relu, bias=b1b[:])

        # raw^T = (W2^T @ hid + b2) * vs   [224, SPC]
        pr_a = mpsum.tile([128, SPC], f32, tag="mp")
        nc.tensor.matmul(pr_a[:], lhsT=w2a[:, 0:128], rhs=hida[:], start=True, stop=False)
        nc.tensor.matmul(pr_a[:], lhsT=w2b[:, 0:128], rhs=hidb[:], start=False, stop=True)
        rawa = spool.tile([128, SPC], f32)
        nc.scalar.activation(rawa[:], pr_a[:], func=AF.Identity, bias=b2v_a[:], scale=WSCALE)
        pr_b = mpsum.tile([96, SPC], f32, tag="mp")
        nc.tensor.matmul(pr_b[:], lhsT=w2a[:, 128:224], rhs=hida[:], start=True, stop=False)
        nc.tensor.matmul(pr_b[:], lhsT=w2b[:, 128:224], rhs=hidb[:], start=False, stop=True)
        rawb = spool.tile([96, SPC], f32)
        nc.scalar.activation(rawb[:], pr_b[:], func=AF.Identity, bias=b2v_b[:], scale=WSCALE)

        nc.sync.dma_start(rawT_d.ap()[0:128, :], rawa[:])
        nc.sync.dma_start(rawT_d.ap()[128:224, :], rawb[:])

        # conv1 stationary: block-diag [4*27, 4*8] bf16, partition order
        # (ky, s, ci, kx): lhsT1[ky*36 + s*9 + ci*3 + kx, s*8 + t] = wt[s][t,ci,ky,kx]
        lhsT1 = cpool.tile([SPC * K27, SPC * TMP], bf16)
        nc.vector.memset(lhsT1[:], 0.0)
        for s in range(SPC):
            for ky in range(K):
                for ci in range(CIN):
                    src = AP(
                        tensor=rawT_d,
                        offset=(ci * K * K + ky * K) * SPC + s,
                        ap=[[SPC, K], [K27 * SPC, TMP]],
                    )
                    p0 = ky * 36 + s * 9 + ci * K
                    nc.gpsimd.dma_start(
                        lhsT1[p0:p0 + K, s * TMP:(s + 1) * TMP], src
                    )
        # conv1 bias vector [32, 1]: bias1[s*8+t] = rawT[216+t, s]
        bias1 = cpool.tile([SPC * TMP, 1], f32)
        for s in range(SPC):
            nc.sync.dma_start(
                bias1[s * TMP:(s + 1) * TMP, :], rawT_d.ap()[216:224, s:s + 1]
            )
        # bias rows need scale 0.1, not 0.1/27 -> multiply by 27
        nc.vector.tensor_scalar_mul(bias1[:], bias1[:], float(K27))

        # ================= Stage B prep: padded bf16 planes =================
        # cast whole padded X (incl. zero ring + tail) to bf16
        nc.gpsimd.dma_start(padXb.ap(), padX.ap())
        # padY lives in SBUF: planes (s,t) on 32 partitions, PP bf16 each.
        # conv1 epilogue (ACT) writes the interior directly; zero the ring once.
        ypool_res = ctx.enter_context(tc.tile_pool(name="ypres", bufs=1))
        padY_sb = ypool_res.tile([SPC * TMP, PP], bf16)
        nc.vector.memset(padY_sb[:, 0:PW], 0.0)                    # top row
        nc.vector.memset(padY_sb[:, 225 * PW:PP], 0.0)             # bottom row + tail
        lr = padY_sb[:, PW:225 * PW].rearrange("p (r c) -> p r c", c=PW)
        nc.vector.memset(lr[:, :, 0:1], 0.0)                       # left col
        nc.vector.memset(lr[:, :, 225:226], 0.0)                   # right col

        # ================= Stage B: conv pipeline =================
        ic1 = ctx.enter_context(tc.tile_pool(name="ic1", bufs=2))
        ic2 = ctx.enter_context(tc.tile_pool(name="ic2", bufs=3))
        op_ = ctx.enter_context(tc.tile_pool(name="opool", bufs=3))
        ps1 = ctx.enter_context(tc.tile_pool(name="ps1", bufs=2, space="PSUM"))
        ps2 = ctx.enter_context(tc.tile_pool(name="ps2", bufs=2, space="PSUM"))

        def conv1_iter(r):
            r0 = r * RT
            t1 = ic1.tile([SPC * K27, RT, PW], bf16, name=f"t1_{r}", tag="t1")
            # partition (ky, s, ci, kx) <- padXb plane (s,ci), shifted by ky*PW+kx
            for ky in range(K):
                src = AP(
                    tensor=padXb,
                    offset=(r0 + ky) * PW,
                    ap=[[PP, SPC * CIN], [1, K], [1, RT * PW]],
                )
                nc.sync.dma_start(t1[ky * 36:(ky + 1) * 36], src)
            for j in range(MM_PER_TILE):
                p1 = ps1.tile([SPC * TMP, NFREE], f32, name=f"p1_{r}_{j}", tag="p1")
                nc.tensor.matmul(
                    p1[:], lhsT=lhsT1[:], rhs=t1[:, j * RMM:(j + 1) * RMM, 0:W],
                    start=True, stop=True,
                )
                # write y rows (r0+2j, r0+2j+1) straight into padY_sb interior
                dst = AP(
                    tensor=padY_sb.tensor,
                    offset=(1 + r0 + j * RMM) * PW + 1,
                    ap=[[PP, SPC * TMP], [PW, RMM], [1, W]],
                )
                nc.scalar.activation(
                    dst, p1[:].rearrange("p (r c) -> p r c", c=W),
                    func=AF.Identity, bias=bias1[:],
                )

        ep_ctr = [0]

        def conv2_iter(s, r):
            r0 = r * RT
            t2 = ic2.tile([K72, RT, PW], bf16, name=f"t2_{s}_{r}", tag="t2")
            # partition (dy, t, dx) <- padY_sb plane (s,t), shifted by dy*PW+dx
            for dy in range(K):
                src = AP(
                    tensor=padY_sb.tensor,
                    offset=s * TMP * PP + (r0 + dy) * PW,
                    ap=[[PP, TMP], [1, K], [1, RT * PW]],
                )
                nc.sync.dma_start(t2[dy * 24:(dy + 1) * 24], src)
            osb = op_.tile([COUT, RT * W], bf16, name=f"o_{s}_{r}", tag="o")
            for jp in range(MM_PER_TILE // 2):
                # two matmuls into the two banks of one [64, 1024] psum tile
                p2 = ps2.tile([COUT, 1024], f32, name=f"p2_{s}_{r}_{jp}", tag="p2")
                nc.tensor.matmul(
                    p2[:, 0:NFREE], lhsT=lhsT2[:],
                    rhs=t2[:, 4 * jp:4 * jp + 2, 0:W], start=True, stop=True,
                )
                nc.tensor.matmul(
                    p2[:, 512:512 + NFREE], lhsT=lhsT2[:],
                    rhs=t2[:, 4 * jp + 2:4 * jp + 4, 0:W], start=True, stop=True,
                )
                pv = p2.rearrange("p (a b) -> p a b", a=2)[:, :, 0:NFREE]
                oslice = osb[:, jp * 2 * NFREE:(jp + 1) * 2 * NFREE].rearrange(
                    "p (a b) -> p a b", a=2)
                if ep_ctr[0] % 5 < 2:  # ~40% on ScalarE, rest on VectorE
                    nc.scalar.activation(oslice, pv, func=AF.Relu, bias=cnnb_sb[:])
                else:
                    nc.vector.tensor_scalar(
                        oslice, pv, cnnb_sb[:], 0.0, op0=ALU.add, op1=ALU.max
                    )
                ep_ctr[0] += 1
            # bf16 -> f32 upcast during store (SWDGE)
            nc.gpsimd.dma_start(out.ap()[s, :, r0 * W:(r0 + RT) * W], osb[:])

        conv1_iter(0)
        conv1_iter(1)
        for r in range(NRT):
            if r + 2 < NRT:
                conv1_iter(r + 2)
            for s in range(SPC):
                conv2_iter(s, r)

    nc.compile()
    _CACHE["nc"] = nc
    return nc


def make_in_maps(X, flat_x, W1, b1, W2, b2, cnn_w, cnn_b):
    X = np.asarray(X, np.float32)
    flat_x = np.asarray(flat_x, np.float32)
    W1 = np.asarray(W1, np.float32)
    b1 = np.asarray(b1, np.float32)
    W2 = np.asarray(W2, np.float32)
    b2 = np.asarray(b2, np.float32)
    cnn_w = np.asarray(cnn_w, np.float32)
    cnn_b = np.asarray(cnn_b, np.float32)

    img = np.zeros((B, CIN, PH, PW), np.float32)
    img[:, :, 1:1 + H, 1:1 + W] = X
    Xp = np.zeros((B, CIN, PP), np.float32)
    Xp[:, :, :PLANE] = img.reshape(B, CIN, PLANE)
    fxT_full = np.ascontiguousarray(flat_x.T)                  # [128, 32]
    cnn_wT = np.ascontiguousarray(
        cnn_w.transpose(2, 1, 3, 0).reshape(K72, COUT))        # [72,64] (dy,t,dx,co)

    in_maps = []
    for i in range(NCORES):
        sl = slice(i * SPC, (i + 1) * SPC)
        in_maps.append({
            "padX": np.ascontiguousarray(Xp[sl].reshape(SPC * CIN, PP)),
            "fxT": np.ascontiguousarray(fxT_full[:, sl]),
            "W1": W1, "b1": b1, "W2": W2, "b2": b2,
            "cnn_wT": cnn_wT, "cnn_b": cnn_b,
        })
    return in_maps


def kernel(X, flat_x, W1, b1, W2, b2, cnn_w, cnn_b):
    nc = build_module()
    in_maps = make_in_maps(X, flat_x, W1, b1, W2, b2, cnn_w, cnn_b)
    res = run_bass_kernel_spmd(nc, in_maps, core_ids=list(range(NCORES)))
    outs = [res.results[i]["out"].reshape(SPC, COUT, H, W) for i in range(NCORES)]
    return np.concatenate(outs, axis=0).astype(np.float32)


# revision 10
# speedup vs baseline: 48.6851x; 48.6851x over previous
"""Trainium2 Bass kernel for nn_MergeMetaCNN (hypernetwork MLP -> grouped conv -> CNN).

Data-parallel over batch: 32 samples -> 8 NeuronCores, 4 samples each.

Per-core pipeline (all math on device):
  1. MLP (fp32 matmuls): hid = relu(W1^T fxT + b1); rawT = W2^T hid + b2,
     scaled per-row by 0.1/27 (filter part) / 0.1 (bias part).
  2. conv1 (grouped 3x3, per-sample dynamic filters) as ONE matmul pass:
     block-diagonal stationary [4*27, 4*8] (bf16), moving operand = im2col
     tile [108, rows, 226] DMA-gathered from zero-padded bf16 X planes
     (each partition = contiguous shifted window of a padded plane).
  3. conv2 (8->64, 3x3) per sample: stationary [72, 64], moving = im2col
     [72, rows, 226] gathered from padded bf16 y planes.
  4. Epilogue relu(x + b) split across ScalarE/VectorE, bf16 staging,
     upcast to fp32 during the store DMA (SWDGE cast).
"""

import numpy as np
from contextlib import ExitStack

import concourse.bass as bass
import concourse.tile as tile
from concourse import bacc, mybir
from concourse.bass_utils import run_bass_kernel_spmd

AP = bass.AP
f32 = mybir.dt.float32
bf16 = mybir.dt.bfloat16
AF = mybir.ActivationFunctionType
ALU = mybir.AluOpType

# Problem constants (hardcoded per contract)
B, CIN, H, W = 32, 3, 224, 224
TMP, K, FLAT, COUT = 8, 3, 128, 64
MLP_OUT = TMP * CIN * K * K + TMP  # 224
META = 0.1
NCORES = 8
SPC = B // NCORES                  # 4 samples per core
PH, PW = H + 2, W + 2              # 226 (zero-pad 1 on each side)
PLANE = PH * PW                    # 51076
PP = PLANE + 4                     # padded plane stride (tail slack for windows)
K27 = CIN * K * K                  # 27
K72 = TMP * K * K                  # 72
RT = 16                            # image rows per row-tile
NRT = H // RT                      # 14 row-tiles
RMM = 2                            # rows per matmul (PSUM bank: 448 fp32 <= 512)
NFREE = RMM * W                    # 448
MM_PER_TILE = RT // RMM            # 4

_CACHE = {}


def build_module(repeat=1):
    """Build + compile the single-core Bass module (SPMD across 8 cores).

    repeat>1 duplicates the conv pipeline (timing probe: device time per
    pipeline = slope of wall-clock vs repeat)."""
    key = ("nc", repeat)
    if key in _CACHE:
        return _CACHE[key]
    nc = bacc.Bacc("TRN2", target_bir_lowering=False, debug=False, num_devices=NCORES)

    # ---- DRAM I/O (per-core shapes) ----
    padX = nc.dram_tensor("padX", [SPC * CIN, PP], f32, kind="ExternalInput")
    fxT = nc.dram_tensor("fxT", [FLAT, SPC], f32, kind="ExternalInput")
    W1 = nc.dram_tensor("W1", [FLAT, MLP_OUT], f32, kind="ExternalInput")
    b1 = nc.dram_tensor("b1", [MLP_OUT], f32, kind="ExternalInput")
    W2 = nc.dram_tensor("W2", [MLP_OUT, MLP_OUT], f32, kind="ExternalInput")
    b2 = nc.dram_tensor("b2", [MLP_OUT], f32, kind="ExternalInput")
    cnn_wT = nc.dram_tensor("cnn_wT", [K72, COUT], f32, kind="ExternalInput")
    cnn_b = nc.dram_tensor("cnn_b", [COUT], f32, kind="ExternalInput")
    out = nc.dram_tensor("out", [SPC, COUT, H * W], f32, kind="ExternalOutput")

    padXb = nc.dram_tensor("padXb", [SPC * CIN, PP], bf16)   # bf16 cast of padX
    rawT_d = nc.dram_tensor("rawT_d", [MLP_OUT, SPC], f32)   # MLP out scratch

    with tile.TileContext(nc) as tc, ExitStack() as ctx:
        cpool = ctx.enter_context(tc.tile_pool(name="consts", bufs=1))
        spool = ctx.enter_context(tc.tile_pool(name="stageA", bufs=1))
        mpsum = ctx.enter_context(tc.tile_pool(name="mlp_psum", bufs=2, space="PSUM"))

        # ================= Stage A: MLP + weight prep =================
        w1sb = cpool.tile([FLAT, MLP_OUT], f32)
        nc.sync.dma_start(w1sb[:], W1.ap())
        w2a = cpool.tile([128, MLP_OUT], f32)
        nc.sync.dma_start(w2a[:], W2.ap()[0:128, :])
        w2b = cpool.tile([96, MLP_OUT], f32)
        nc.sync.dma_start(w2b[:], W2.ap()[128:224, :])
        fx_sb = cpool.tile([FLAT, SPC], f32)
        nc.sync.dma_start(fx_sb[:], fxT.ap())
        b1a = cpool.tile([128, 1], f32)
        nc.sync.dma_start(b1a[:], b1.ap()[0:128].unsqueeze(1))
        b1b = cpool.tile([96, 1], f32)
        nc.sync.dma_start(b1b[:], b1.ap()[128:224].unsqueeze(1))
        b2a = cpool.tile([128, 1], f32)
        nc.sync.dma_start(b2a[:], b2.ap()[0:128].unsqueeze(1))
        b2b = cpool.tile([96, 1], f32)
        nc.sync.dma_start(b2b[:], b2.ap()[128:224].unsqueeze(1))
        cnnb_sb = cpool.tile([COUT, 1], f32)
        nc.sync.dma_start(cnnb_sb[:], cnn_b.ap().unsqueeze(1))
        lhsT2 = cpool.tile([K72, COUT], bf16)
        nc.gpsimd.dma_start(lhsT2[:], cnn_wT.ap())  # cast f32 -> bf16

        # uniform scale 0.1/27 on all raw rows; bias rows corrected by x27 later
        WSCALE = META / K27
        b2v_a = cpool.tile([128, 1], f32)
        nc.vector.tensor_scalar_mul(b2v_a[:], b2a[:], WSCALE)
        b2v_b = cpool.tile([96, 1], f32)
        nc.vector.tensor_scalar_mul(b2v_b[:], b2b[:], WSCALE)

        # hid^T = relu(W1^T @ fxT + b1)   [224, SPC] in two partition chunks
        ph_a = mpsum.tile([128, SPC], f32, tag="mp")
        nc.tensor.matmul(ph_a[:], lhsT=w1sb[:, 0:128], rhs=fx_sb[:], start=True, stop=True)
        hida = spool.tile([128, SPC], f32)
        nc.scalar.activation(hida[:], ph_a[:], func=AF.Relu, bias=b1a[:])
        ph_b = mpsum.tile([96, SPC], f32, tag="mp")
        nc.tensor.matmul(ph_b[:], lhsT=w1sb[:, 128:224], rhs=fx_sb[:], start=True, stop=True)
        hidb = spool.tile([96, SPC], f32)
        nc.scalar.activation(hidb[:], ph_b[:], func=AF.Relu, bias=b1b[:])

        # raw^T = (W2^T @ hid + b2) * vs   [224, SPC]
        pr_a = mpsum.tile([128, SPC], f32, tag="mp")
        nc.tensor.matmul(pr_a[:], lhsT=w2a[:, 0:128], rhs=hida[:], start=True, stop=False)
        nc.tensor.matmul(pr_a[:], lhsT=w2b[:, 0:128], rhs=hidb[:], start=False, stop=True)
        rawa = spool.tile([128, SPC], f32)
        nc.scalar.activation(rawa[:], pr_a[:], func=AF.Identity, bias=b2v_a[:], scale=WSCALE)
        pr_b = mpsum.tile([96, SPC], f32, tag="mp")
        nc.tensor.matmul(pr_b[:], lhsT=w2a[:, 128:224], rhs=hida[:], start=True, stop=False)
        nc.tensor.matmul(pr_b[:], lhsT=w2b[:, 128:224], rhs=hidb[:], start=False, stop=True)
        rawb = spool.tile([96, SPC], f32)
        nc.scalar.activation(rawb[:], pr_b[:], func=AF.Identity, bias=b2v_b[:], scale=WSCALE)

        nc.sync.dma_start(rawT_d.ap()[0:128, :], rawa[:])
        nc.sync.dma_start(rawT_d.ap()[128:224, :], rawb[:])

        # conv1 stationary: block-diag [4*27, 4*8] bf16, partition order
        # (ky, s, ci, kx): lhsT1[ky*36 + s*9 + ci*3 + kx, s*8 + t] = wt[s][t,ci,ky,kx]
        lhsT1 = cpool.tile([SPC * K27, SPC * TMP], bf16)
        nc.vector.memset(lhsT1[:], 0.0)
        for s in range(SPC):
            for ky in range(K):
                for ci in range(CIN):
                    src = AP(
                        tensor=rawT_d,
                        offset=(ci * K * K + ky * K) * SPC + s,
                        ap=[[SPC, K], [K27 * SPC, TMP]],
                    )
                    p0 = ky * 36 + s * 9 + ci * K
                    nc.gpsimd.dma_start(
                        lhsT1[p0:p0 + K, s * TMP:(s + 1) * TMP], src
                    )
        # conv1 bias vector [32, 1]: bias1[s*8+t] = rawT[216+t, s]
        bias1 = cpool.tile([SPC * TMP, 1], f32)
        for s in range(SPC):
            nc.sync.dma_start(
                bias1[s * TMP:(s + 1) * TMP, :], rawT_d.ap()[216:224, s:s + 1]
            )
        # bias rows need scale 0.1, not 0.1/27 -> multiply by 27
        nc.vector.tensor_scalar_mul(bias1[:], bias1[:], float(K27))

        # ================= Stage B prep: padded bf16 planes =================
        # cast whole padded X (incl. zero ring + tail) to bf16
        nc.gpsimd.dma_start(padXb.ap(), padX.ap())
        # padY lives in SBUF: planes (s,t) on 32 partitions, PP bf16 each.
        # conv1 epilogue (ACT) writes the interior directly; zero the ring once.
        ypool_res = ctx.enter_context(tc.tile_pool(name="ypres", bufs=1))
        padY_sb = ypool_res.tile([SPC * TMP, PP], bf16)
        nc.vector.memset(padY_sb[:, 0:PW], 0.0)                    # top row
        nc.vector.memset(padY_sb[:, 225 * PW:PP], 0.0)             # bottom row + tail
        lr = padY_sb[:, PW:225 * PW].rearrange("p (r c) -> p r c", c=PW)
        nc.vector.memset(lr[:, :, 0:1], 0.0)                       # left col
        nc.vector.memset(lr[:, :, 225:226], 0.0)                   # right col

        # ================= Stage B: conv pipeline =================
        ic1 = ctx.enter_context(tc.tile_pool(name="ic1", bufs=2))
        ic2 = ctx.enter_context(tc.tile_pool(name="ic2", bufs=3))
        op_ = ctx.enter_context(tc.tile_pool(name="opool", bufs=3))
        ps1 = ctx.enter_context(tc.tile_pool(name="ps1", bufs=2, space="PSUM"))
        ps2 = ctx.enter_context(tc.tile_pool(name="ps2", bufs=2, space="PSUM"))

        def conv1_iter(r):
            r0 = r * RT
            t1 = ic1.tile([SPC * K27, RT, PW], bf16, name=f"t1_{ep_ctr[0]}_{r}", tag="t1")
            # partition (ky, s, ci, kx) <- padXb plane (s,ci), shifted by ky*PW+kx
            for ky in range(K):
                src = AP(
                    tensor=padXb,
                    offset=(r0 + ky) * PW,
                    ap=[[PP, SPC * CIN], [1, K], [1, RT * PW]],
                )
                nc.sync.dma_start(t1[ky * 36:(ky + 1) * 36], src)
            for j in range(MM_PER_TILE):
                p1 = ps1.tile([SPC * TMP, NFREE], f32, name=f"p1_{ep_ctr[0]}_{r}_{j}", tag="p1")
                nc.tensor.matmul(
                    p1[:], lhsT=lhsT1[:], rhs=t1[:, j * RMM:(j + 1) * RMM, 0:W],
                    start=True, stop=True,
                )
                # write y rows (r0+2j, r0+2j+1) straight into padY_sb interior
                dst = AP(
                    tensor=padY_sb.tensor,
                    offset=(1 + r0 + j * RMM) * PW + 1,
                    ap=[[PP, SPC * TMP], [PW, RMM], [1, W]],
                )
                nc.scalar.activation(
                    dst, p1[:].rearrange("p (r c) -> p r c", c=W),
                    func=AF.Identity, bias=bias1[:],
                )

        ep_ctr = [0]

        def conv2_iter(s, r):
            r0 = r * RT
            t2 = ic2.tile([K72, RT, PW], bf16, name=f"t2_{ep_ctr[0]}_{s}_{r}", tag="t2")
            # partition (dy, t, dx) <- padY_sb plane (s,t), shifted by dy*PW+dx
            for dy in range(K):
                src = AP(
                    tensor=padY_sb.tensor,
                    offset=s * TMP * PP + (r0 + dy) * PW,
                    ap=[[PP, TMP], [1, K], [1, RT * PW]],
                )
                nc.sync.dma_start(t2[dy * 24:(dy + 1) * 24], src)
            osb = op_.tile([COUT, RT * W], bf16, name=f"o_{ep_ctr[0]}_{s}_{r}", tag="o")
            for jp in range(MM_PER_TILE // 2):
                # two matmuls into the two banks of one [64, 1024] psum tile
                p2 = ps2.tile([COUT, 1024], f32, name=f"p2_{ep_ctr[0]}_{s}_{r}_{jp}", tag="p2")
                nc.tensor.matmul(
                    p2[:, 0:NFREE], lhsT=lhsT2[:],
                    rhs=t2[:, 4 * jp:4 * jp + 2, 0:W], start=True, stop=True,
                )
                nc.tensor.matmul(
                    p2[:, 512:512 + NFREE], lhsT=lhsT2[:],
                    rhs=t2[:, 4 * jp + 2:4 * jp + 4, 0:W], start=True, stop=True,
                )
                pv = p2.rearrange("p (a b) -> p a b", a=2)[:, :, 0:NFREE]
                oslice = osb[:, jp * 2 * NFREE:(jp + 1) * 2 * NFREE].rearrange(
                    "p (a b) -> p a b", a=2)
                if ep_ctr[0] % 5 < 2:  # ~40% on ScalarE, rest on VectorE
                    nc.scalar.activation(oslice, pv, func=AF.Relu, bias=cnnb_sb[:])
                else:
                    nc.vector.tensor_scalar(
                        oslice, pv, cnnb_sb[:], 0.0, op0=ALU.add, op1=ALU.max
                    )
                ep_ctr[0] += 1
            # bf16 -> f32 upcast during store (SWDGE)
            nc.gpsimd.dma_start(out.ap()[s, :, r0 * W:(r0 + RT) * W], osb[:])

        for _rep in range(repeat):
            conv1_iter(0)
            conv1_iter(1)
            for r in range(NRT):
                if r + 2 < NRT:
                    conv1_iter(r + 2)
                for s in range(SPC):
                    conv2_iter(s, r)

    nc.compile()
    _CACHE[key] = nc
    return nc


def make_in_maps(X, flat_x, W1, b1, W2, b2, cnn_w, cnn_b):
    X = np.asarray(X, np.float32)
    flat_x = np.asarray(flat_x, np.float32)
    W1 = np.asarray(W1, np.float32)
    b1 = np.asarray(b1, np.float32)
    W2 = np.asarray(W2, np.float32)
    b2 = np.asarray(b2, np.float32)
    cnn_w = np.asarray(cnn_w, np.float32)
    cnn_b = np.asarray(cnn_b, np.float32)

    img = np.zeros((B, CIN, PH, PW), np.float32)
    img[:, :, 1:1 + H, 1:1 + W] = X
    Xp = np.zeros((B, CIN, PP), np.float32)
    Xp[:, :, :PLANE] = img.reshape(B, CIN, PLANE)
    fxT_full = np.ascontiguousarray(flat_x.T)                  # [128, 32]
    cnn_wT = np.ascontiguousarray(
        cnn_w.transpose(2, 1, 3, 0).reshape(K72, COUT))        # [72,64] (dy,t,dx,co)

    in_maps = []
    for i in range(NCORES):
        sl = slice(i * SPC, (i + 1) * SPC)
        in_maps.append({
            "padX": np.ascontiguousarray(Xp[sl].reshape(SPC * CIN, PP)),
            "fxT": np.ascontiguousarray(fxT_full[:, sl]),
            "W1": W1, "b1": b1, "W2": W2, "b2": b2,
            "cnn_wT": cnn_wT, "cnn_b": cnn_b,
        })
    return in_maps


def kernel(X, flat_x, W1, b1, W2, b2, cnn_w, cnn_b):
    nc = build_module()
    in_maps = make_in_maps(X, flat_x, W1, b1, W2, b2, cnn_w, cnn_b)
    res = run_bass_kernel_spmd(nc, in_maps, core_ids=list(range(NCORES)))
    outs = [res.results[i]["out"].reshape(SPC, COUT, H, W) for i in range(NCORES)]
    return np.concatenate(outs, axis=0).astype(np.float32)


# revision 11
# speedup vs baseline: 74.9520x; 1.5395x over previous
"""Trainium2 Bass kernel for nn_MergeMetaCNN (hypernetwork MLP -> grouped conv -> CNN).

Data-parallel over batch: 32 samples -> 8 NeuronCores, 4 samples each.

Per-core pipeline (all math on device):
  1. MLP (fp32 matmuls): hid = relu(W1^T fxT + b1); rawT = W2^T hid + b2,
     scaled per-row by 0.1/27 (filter part) / 0.1 (bias part).
  2. conv1 (grouped 3x3, per-sample dynamic filters) as ONE matmul pass:
     block-diagonal stationary [4*27, 4*8] (bf16), moving operand = im2col
     tile [108, rows, 226] DMA-gathered from zero-padded bf16 X planes
     (each partition = contiguous shifted window of a padded plane).
  3. conv2 (8->64, 3x3) per sample: stationary [72, 64], moving = im2col
     [72, rows, 226] gathered from padded bf16 y planes.
  4. Epilogue relu(x + b) split across ScalarE/VectorE, bf16 staging,
     upcast to fp32 during the store DMA (SWDGE cast).
"""

import numpy as np
from contextlib import ExitStack

import concourse.bass as bass
import concourse.tile as tile
from concourse import bacc, mybir
from concourse.bass_utils import run_bass_kernel_spmd

AP = bass.AP
f32 = mybir.dt.float32
bf16 = mybir.dt.bfloat16
AF = mybir.ActivationFunctionType
ALU = mybir.AluOpType

# Problem constants (hardcoded per contract)
B, CIN, H, W = 32, 3, 224, 224
TMP, K, FLAT, COUT = 8, 3, 128, 64
MLP_OUT = TMP * CIN * K * K + TMP  # 224
META = 0.1
NCORES = 8
SPC = B // NCORES                  # 4 samples per core
PH, PW = H + 2, W + 2              # 226 (zero-pad 1 on each side)
PLANE = PH * PW                    # 51076
PP = PLANE + 4                     # padded plane stride (tail slack for windows)
K27 = CIN * K * K                  # 27
K72 = TMP * K * K                  # 72
RT = 16                            # image rows per row-tile
NRT = H // RT                      # 14 row-tiles
RMM = 2                            # rows per matmul (PSUM bank: 448 fp32 <= 512)
NFREE = RMM * W                    # 448
MM_PER_TILE = RT // RMM            # 4

_CACHE = {}


def build_module(repeat=1):
    """Build + compile the single-core Bass module (SPMD across 8 cores).

    repeat>1 duplicates the conv pipeline (timing probe: device time per
    pipeline = slope of wall-clock vs repeat)."""
    key = ("nc", repeat)
    if key in _CACHE:
        return _CACHE[key]
    nc = bacc.Bacc("TRN2", target_bir_lowering=False, debug=False, num_devices=NCORES)

    # ---- DRAM I/O (per-core shapes) ----
    padX = nc.dram_tensor("padX", [SPC * CIN, PP], f32, kind="ExternalInput")
    fxT = nc.dram_tensor("fxT", [FLAT, SPC], f32, kind="ExternalInput")
    W1 = nc.dram_tensor("W1", [FLAT, MLP_OUT], f32, kind="ExternalInput")
    b1 = nc.dram_tensor("b1", [MLP_OUT], f32, kind="ExternalInput")
    W2 = nc.dram_tensor("W2", [MLP_OUT, MLP_OUT], f32, kind="ExternalInput")
    b2 = nc.dram_tensor("b2", [MLP_OUT], f32, kind="ExternalInput")
    cnn_wT = nc.dram_tensor("cnn_wT", [K72, COUT], f32, kind="ExternalInput")
    cnn_b = nc.dram_tensor("cnn_b", [COUT], f32, kind="ExternalInput")
    out = nc.dram_tensor("out", [SPC, COUT, H * W], f32, kind="ExternalOutput")

    padXb = nc.dram_tensor("padXb", [SPC * CIN, PP], bf16)   # bf16 cast of padX
    rawT_d = nc.dram_tensor("rawT_d", [MLP_OUT, SPC], f32)   # MLP out scratch

    with tile.TileContext(nc) as tc, ExitStack() as ctx:
        cpool = ctx.enter_context(tc.tile_pool(name="consts", bufs=1))
        spool = ctx.enter_context(tc.tile_pool(name="stageA", bufs=1))
        mlp_ctx = ExitStack()
        mpsum = mlp_ctx.enter_context(tc.tile_pool(name="mlp_psum", bufs=2, space="PSUM"))

        # ================= Stage A: MLP + weight prep =================
        w1sb = cpool.tile([FLAT, MLP_OUT], f32)
        nc.sync.dma_start(w1sb[:], W1.ap())
        w2a = cpool.tile([128, MLP_OUT], f32)
        nc.sync.dma_start(w2a[:], W2.ap()[0:128, :])
        w2b = cpool.tile([96, MLP_OUT], f32)
        nc.sync.dma_start(w2b[:], W2.ap()[128:224, :])
        fx_sb = cpool.tile([FLAT, SPC], f32)
        nc.sync.dma_start(fx_sb[:], fxT.ap())
        b1a = cpool.tile([128, 1], f32)
        nc.sync.dma_start(b1a[:], b1.ap()[0:128].unsqueeze(1))
        b1b = cpool.tile([96, 1], f32)
        nc.sync.dma_start(b1b[:], b1.ap()[128:224].unsqueeze(1))
        b2a = cpool.tile([128, 1], f32)
        nc.sync.dma_start(b2a[:], b2.ap()[0:128].unsqueeze(1))
        b2b = cpool.tile([96, 1], f32)
        nc.sync.dma_start(b2b[:], b2.ap()[128:224].unsqueeze(1))
        cnnb_sb = cpool.tile([COUT, 1], f32)
        nc.sync.dma_start(cnnb_sb[:], cnn_b.ap().unsqueeze(1))
        lhsT2 = cpool.tile([K72, COUT], bf16)
        nc.gpsimd.dma_start(lhsT2[:], cnn_wT.ap())  # cast f32 -> bf16

        # uniform scale 0.1/27 on all raw rows; bias rows corrected by x27 later
        WSCALE = META / K27
        b2v_a = cpool.tile([128, 1], f32)
        nc.vector.tensor_scalar_mul(b2v_a[:], b2a[:], WSCALE)
        b2v_b = cpool.tile([96, 1], f32)
        nc.vector.tensor_scalar_mul(b2v_b[:], b2b[:], WSCALE)

        # hid^T = relu(W1^T @ fxT + b1)   [224, SPC] in two partition chunks
        ph_a = mpsum.tile([128, SPC], f32, tag="mp")
        nc.tensor.matmul(ph_a[:], lhsT=w1sb[:, 0:128], rhs=fx_sb[:], start=True, stop=True)
        hida = spool.tile([128, SPC], f32)
        nc.scalar.activation(hida[:], ph_a[:], func=AF.Relu, bias=b1a[:])
        ph_b = mpsum.tile([96, SPC], f32, tag="mp")
        nc.tensor.matmul(ph_b[:], lhsT=w1sb[:, 128:224], rhs=fx_sb[:], start=True, stop=True)
        hidb = spool.tile([96, SPC], f32)
        nc.scalar.activation(hidb[:], ph_b[:], func=AF.Relu, bias=b1b[:])

        # raw^T = (W2^T @ hid + b2) * vs   [224, SPC]
        pr_a = mpsum.tile([128, SPC], f32, tag="mp")
        nc.tensor.matmul(pr_a[:], lhsT=w2a[:, 0:128], rhs=hida[:], start=True, stop=False)
        nc.tensor.matmul(pr_a[:], lhsT=w2b[:, 0:128], rhs=hidb[:], start=False, stop=True)
        rawa = spool.tile([128, SPC], f32)
        nc.scalar.activation(rawa[:], pr_a[:], func=AF.Identity, bias=b2v_a[:], scale=WSCALE)
        pr_b = mpsum.tile([96, SPC], f32, tag="mp")
        nc.tensor.matmul(pr_b[:], lhsT=w2a[:, 128:224], rhs=hida[:], start=True, stop=False)
        nc.tensor.matmul(pr_b[:], lhsT=w2b[:, 128:224], rhs=hidb[:], start=False, stop=True)
        rawb = spool.tile([96, SPC], f32)
        nc.scalar.activation(rawb[:], pr_b[:], func=AF.Identity, bias=b2v_b[:], scale=WSCALE)

        nc.sync.dma_start(rawT_d.ap()[0:128, :], rawa[:])
        nc.sync.dma_start(rawT_d.ap()[128:224, :], rawb[:])

        # conv1 stationary: block-diag [4*27, 4*8] bf16, partition order
        # (ky, s, ci, kx): lhsT1[ky*36 + s*9 + ci*3 + kx, s*8 + t] = wt[s][t,ci,ky,kx]
        lhsT1 = cpool.tile([SPC * K27, SPC * TMP], bf16)
        nc.vector.memset(lhsT1[:], 0.0)
        for s in range(SPC):
            for ky in range(K):
                for ci in range(CIN):
                    src = AP(
                        tensor=rawT_d,
                        offset=(ci * K * K + ky * K) * SPC + s,
                        ap=[[SPC, K], [K27 * SPC, TMP]],
                    )
                    p0 = ky * 36 + s * 9 + ci * K
                    nc.gpsimd.dma_start(
                        lhsT1[p0:p0 + K, s * TMP:(s + 1) * TMP], src
                    )
        # conv1 bias vector [32, 1]: bias1[s*8+t] = rawT[216+t, s]
        bias1 = cpool.tile([SPC * TMP, 1], f32)
        for s in range(SPC):
            nc.sync.dma_start(
                bias1[s * TMP:(s + 1) * TMP, :], rawT_d.ap()[216:224, s:s + 1]
            )
        # bias rows need scale 0.1, not 0.1/27 -> multiply by 27
        nc.vector.tensor_scalar_mul(bias1[:], bias1[:], float(K27))

        mlp_ctx.close()  # release MLP PSUM banks for conv pools

        # ================= Stage B prep: padded bf16 planes =================
        # cast whole padded X (incl. zero ring + tail) to bf16
        nc.gpsimd.dma_start(padXb.ap(), padX.ap())
        # padY lives in SBUF: planes (s,t) on 32 partitions, PP bf16 each.
        # conv1 epilogue (ACT) writes the interior directly; zero the ring once.
        ypool_res = ctx.enter_context(tc.tile_pool(name="ypres", bufs=1))
        padY_sb = ypool_res.tile([SPC * TMP, PP], bf16)
        nc.vector.memset(padY_sb[:, 0:PW], 0.0)                    # top row
        nc.vector.memset(padY_sb[:, 225 * PW:PP], 0.0)             # bottom row + tail
        lr = padY_sb[:, PW:225 * PW].rearrange("p (r c) -> p r c", c=PW)
        nc.vector.memset(lr[:, :, 0:1], 0.0)                       # left col
        nc.vector.memset(lr[:, :, 225:226], 0.0)                   # right col

        # ================= Stage B: conv pipeline =================
        ic1 = ctx.enter_context(tc.tile_pool(name="ic1", bufs=2))
        ic2 = ctx.enter_context(tc.tile_pool(name="ic2", bufs=3))
        op_ = ctx.enter_context(tc.tile_pool(name="opool", bufs=2))
        ps1 = ctx.enter_context(tc.tile_pool(name="ps1", bufs=2, space="PSUM"))
        ps2 = ctx.enter_context(tc.tile_pool(name="ps2", bufs=3, space="PSUM"))

        def conv1_iter(r):
            r0 = r * RT
            t1 = ic1.tile([SPC * K27, RT, PW], bf16, name=f"t1_{ep_ctr[0]}_{r}", tag="t1")
            # partition (ky, s, ci, kx) <- padXb plane (s,ci), shifted by ky*PW+kx
            for ky in range(K):
                src = AP(
                    tensor=padXb,
                    offset=(r0 + ky) * PW,
                    ap=[[PP, SPC * CIN], [1, K], [1, RT * PW]],
                )
                nc.sync.dma_start(t1[ky * 36:(ky + 1) * 36], src)
            for j in range(MM_PER_TILE):
                p1 = ps1.tile([SPC * TMP, NFREE], f32, name=f"p1_{ep_ctr[0]}_{r}_{j}", tag="p1")
                nc.tensor.matmul(
                    p1[:], lhsT=lhsT1[:], rhs=t1[:, j * RMM:(j + 1) * RMM, 0:W],
                    start=True, stop=True,
                )
                # write y rows (r0+2j, r0+2j+1) straight into padY_sb interior
                dst = AP(
                    tensor=padY_sb.tensor,
                    offset=(1 + r0 + j * RMM) * PW + 1,
                    ap=[[PP, SPC * TMP], [PW, RMM], [1, W]],
                )
                nc.scalar.activation(
                    dst, p1[:].rearrange("p (r c) -> p r c", c=W),
                    func=AF.Identity, bias=bias1[:],
                )

        ep_ctr = [0]

        def conv2_pair(s0, r):
            # two samples (s0, s0+1) share one output-store DMA
            r0 = r * RT
            osb = op_.tile([COUT, 2 * RT * W], bf16,
                           name=f"o_{ep_ctr[0]}_{s0}_{r}", tag="o")
            for si in range(2):
                s = s0 + si
                t2 = ic2.tile([K72, RT, PW], bf16,
                              name=f"t2_{ep_ctr[0]}_{s}_{r}", tag="t2")
                # partition (dy, t, dx) <- padY_sb plane (s,t), shifted dy*PW+dx
                for dy in range(K):
                    src = AP(
                        tensor=padY_sb.tensor,
                        offset=s * TMP * PP + (r0 + dy) * PW,
                        ap=[[PP, TMP], [1, K], [1, RT * PW]],
                    )
                    nc.sync.dma_start(t2[dy * 24:(dy + 1) * 24], src)
                for jp in range(MM_PER_TILE // 2):
                    # two matmuls into the two banks of one [64, 1024] psum tile
                    p2 = ps2.tile([COUT, 1024], f32,
                                  name=f"p2_{ep_ctr[0]}_{s}_{r}_{jp}", tag="p2")
                    nc.tensor.matmul(
                        p2[:, 0:NFREE], lhsT=lhsT2[:],
                        rhs=t2[:, 4 * jp:4 * jp + 2, 0:W], start=True, stop=True,
                    )
                    nc.tensor.matmul(
                        p2[:, 512:512 + NFREE], lhsT=lhsT2[:],
                        rhs=t2[:, 4 * jp + 2:4 * jp + 4, 0:W], start=True, stop=True,
                    )
                    pv = p2.rearrange("p (a b) -> p a b", a=2)[:, :, 0:NFREE]
                    obase = si * RT * W + jp * 2 * NFREE
                    oslice = osb[:, obase:obase + 2 * NFREE].rearrange(
                        "p (a b) -> p a b", a=2)
                    if ep_ctr[0] % 5 < 2:  # ~40% ScalarE, rest VectorE
                        nc.scalar.activation(oslice, pv, func=AF.Relu,
                                             bias=cnnb_sb[:])
                    else:
                        nc.vector.tensor_scalar(
                            oslice, pv, cnnb_sb[:], 0.0, op0=ALU.add, op1=ALU.max
                        )
                    ep_ctr[0] += 1
            # bf16 -> f32 upcast during store (SWDGE); both samples, one DMA
            dst = AP(
                tensor=out,
                offset=s0 * COUT * H * W + r0 * W,
                ap=[[H * W, COUT], [COUT * H * W, 2], [1, RT * W]],
            )
            nc.gpsimd.dma_start(dst, osb[:].rearrange("p (a b) -> p a b", a=2))

        for _rep in range(repeat):
            conv1_iter(0)
            conv1_iter(1)
            for r in range(NRT):
                if r + 2 < NRT:
                    conv1_iter(r + 2)
                for s0 in (0, 2):
                    conv2_pair(s0, r)

    nc.compile()
    _CACHE[key] = nc
    return nc


def make_in_maps(X, flat_x, W1, b1, W2, b2, cnn_w, cnn_b):
    X = np.asarray(X, np.float32)
    flat_x = np.asarray(flat_x, np.float32)
    W1 = np.asarray(W1, np.float32)
    b1 = np.asarray(b1, np.float32)
    W2 = np.asarray(W2, np.float32)
    b2 = np.asarray(b2, np.float32)
    cnn_w = np.asarray(cnn_w, np.float32)
    cnn_b = np.asarray(cnn_b, np.float32)

    img = np.zeros((B, CIN, PH, PW), np.float32)
    img[:, :, 1:1 + H, 1:1 + W] = X
    Xp = np.zeros((B, CIN, PP), np.float32)
    Xp[:, :, :PLANE] = img.reshape(B, CIN, PLANE)
    fxT_full = np.ascontiguousarray(flat_x.T)                  # [128, 32]
    cnn_wT = np.ascontiguousarray(
        cnn_w.transpose(2, 1, 3, 0).reshape(K72, COUT))        # [72,64] (dy,t,dx,co)

    in_maps = []
    for i in range(NCORES):
        sl = slice(i * SPC, (i + 1) * SPC)
        in_maps.append({
            "padX": np.ascontiguousarray(Xp[sl].reshape(SPC * CIN, PP)),
            "fxT": np.ascontiguousarray(fxT_full[:, sl]),
            "W1": W1, "b1": b1, "W2": W2, "b2": b2,
            "cnn_wT": cnn_wT, "cnn_b": cnn_b,
        })
    return in_maps


def kernel(X, flat_x, W1, b1, W2, b2, cnn_w, cnn_b):
    nc = build_module()
    in_maps = make_in_maps(X, flat_x, W1, b1, W2, b2, cnn_w, cnn_b)
    res = run_bass_kernel_spmd(nc, in_maps, core_ids=list(range(NCORES)))
    outs = [res.results[i]["out"].reshape(SPC, COUT, H, W) for i in range(NCORES)]
    return np.concatenate(outs, axis=0).astype(np.float32)


# revision 13
# speedup vs baseline: 84.9596x; 1.1335x over previous
"""Trainium2 Bass kernel for nn_MergeMetaCNN (hypernetwork MLP -> grouped conv -> CNN).

Data-parallel over batch: 32 samples -> 8 NeuronCores, 4 samples each.

Per-core pipeline (all math on device):
  1. MLP (fp32 matmuls): hid = relu(W1^T fxT + b1); rawT = W2^T hid + b2,
     scaled per-row by 0.1/27 (filter part) / 0.1 (bias part).
  2. conv1 (grouped 3x3, per-sample dynamic filters) as ONE matmul pass:
     block-diagonal stationary [4*27, 4*8] (bf16), moving operand = im2col
     tile [108, rows, 226] DMA-gathered from zero-padded bf16 X planes
     (each partition = contiguous shifted window of a padded plane).
  3. conv2 (8->64, 3x3) per sample: stationary [72, 64], moving = im2col
     [72, rows, 226] gathered from padded bf16 y planes.
  4. Epilogue relu(x + b) split across ScalarE/VectorE, bf16 staging,
     upcast to fp32 during the store DMA (SWDGE cast).
"""

import numpy as np
from contextlib import ExitStack

import concourse.bass as bass
import concourse.tile as tile
from concourse import bacc, mybir
from concourse.bass_utils import run_bass_kernel_spmd

AP = bass.AP
f32 = mybir.dt.float32
bf16 = mybir.dt.bfloat16
AF = mybir.ActivationFunctionType
ALU = mybir.AluOpType

# Problem constants (hardcoded per contract)
B, CIN, H, W = 32, 3, 224, 224
TMP, K, FLAT, COUT = 8, 3, 128, 64
MLP_OUT = TMP * CIN * K * K + TMP  # 224
META = 0.1
NCORES = 8
SPC = B // NCORES                  # 4 samples per core
PH, PW = H + 2, W + 2              # 226 (zero-pad 1 on each side)
PLANE = PH * PW                    # 51076
PP = PLANE + 4                     # padded plane stride (tail slack for windows)
K27 = CIN * K * K                  # 27
K72 = TMP * K * K                  # 72
RT = 16                            # image rows per row-tile
NRT = H // RT                      # 14 row-tiles
RMM = 2                            # rows per matmul (PSUM bank: 448 fp32 <= 512)
NFREE = RMM * W                    # 448
MM_PER_TILE = RT // RMM            # 4

_CACHE = {}


def build_module(repeat=1):
    """Build + compile the single-core Bass module (SPMD across 8 cores).

    repeat>1 duplicates the conv pipeline (timing probe: device time per
    pipeline = slope of wall-clock vs repeat)."""
    key = ("nc", repeat)
    if key in _CACHE:
        return _CACHE[key]
    nc = bacc.Bacc("TRN2", target_bir_lowering=False, debug=False, num_devices=NCORES)

    # ---- DRAM I/O (per-core shapes) ----
    padX = nc.dram_tensor("padX", [SPC * CIN, PP], f32, kind="ExternalInput")
    fxT = nc.dram_tensor("fxT", [FLAT, SPC], f32, kind="ExternalInput")
    W1 = nc.dram_tensor("W1", [FLAT, MLP_OUT], f32, kind="ExternalInput")
    b1 = nc.dram_tensor("b1", [MLP_OUT], f32, kind="ExternalInput")
    W2 = nc.dram_tensor("W2", [MLP_OUT, MLP_OUT], f32, kind="ExternalInput")
    b2 = nc.dram_tensor("b2", [MLP_OUT], f32, kind="ExternalInput")
    cnn_wT = nc.dram_tensor("cnn_wT", [K72, COUT], f32, kind="ExternalInput")
    cnn_b = nc.dram_tensor("cnn_b", [COUT], f32, kind="ExternalInput")
    out = nc.dram_tensor("out", [SPC, COUT, H * W], f32, kind="ExternalOutput")

    padXb = nc.dram_tensor("padXb", [SPC * CIN, PP], bf16)   # bf16 cast of padX
    rawT_d = nc.dram_tensor("rawT_d", [MLP_OUT, SPC], f32)   # MLP out scratch

    with tile.TileContext(nc) as tc, ExitStack() as ctx:
        cpool = ctx.enter_context(tc.tile_pool(name="consts", bufs=1))
        spool = ctx.enter_context(tc.tile_pool(name="stageA", bufs=1))
        mlp_ctx = ExitStack()
        mpsum = mlp_ctx.enter_context(tc.tile_pool(name="mlp_psum", bufs=2, space="PSUM"))

        # ================= Stage A: MLP + weight prep =================
        w1sb = cpool.tile([FLAT, MLP_OUT], f32)
        nc.sync.dma_start(w1sb[:], W1.ap())
        w2a = cpool.tile([128, MLP_OUT], f32)
        nc.sync.dma_start(w2a[:], W2.ap()[0:128, :])
        w2b = cpool.tile([96, MLP_OUT], f32)
        nc.sync.dma_start(w2b[:], W2.ap()[128:224, :])
        fx_sb = cpool.tile([FLAT, SPC], f32)
        nc.sync.dma_start(fx_sb[:], fxT.ap())
        b1a = cpool.tile([128, 1], f32)
        nc.sync.dma_start(b1a[:], b1.ap()[0:128].unsqueeze(1))
        b1b = cpool.tile([96, 1], f32)
        nc.sync.dma_start(b1b[:], b1.ap()[128:224].unsqueeze(1))
        b2a = cpool.tile([128, 1], f32)
        nc.sync.dma_start(b2a[:], b2.ap()[0:128].unsqueeze(1))
        b2b = cpool.tile([96, 1], f32)
        nc.sync.dma_start(b2b[:], b2.ap()[128:224].unsqueeze(1))
        cnnb_sb = cpool.tile([COUT, 1], f32)
        nc.sync.dma_start(cnnb_sb[:], cnn_b.ap().unsqueeze(1))
        lhsT2 = cpool.tile([K72, COUT], bf16)
        nc.gpsimd.dma_start(lhsT2[:], cnn_wT.ap())  # cast f32 -> bf16

        # uniform scale 0.1/27 on all raw rows; bias rows corrected by x27 later
        WSCALE = META / K27
        b2v_a = cpool.tile([128, 1], f32)
        nc.vector.tensor_scalar_mul(b2v_a[:], b2a[:], WSCALE)
        b2v_b = cpool.tile([96, 1], f32)
        nc.vector.tensor_scalar_mul(b2v_b[:], b2b[:], WSCALE)

        # hid^T = relu(W1^T @ fxT + b1)   [224, SPC] in two partition chunks
        ph_a = mpsum.tile([128, SPC], f32, tag="mp")
        nc.tensor.matmul(ph_a[:], lhsT=w1sb[:, 0:128], rhs=fx_sb[:], start=True, stop=True)
        hida = spool.tile([128, SPC], f32)
        nc.scalar.activation(hida[:], ph_a[:], func=AF.Relu, bias=b1a[:])
        ph_b = mpsum.tile([96, SPC], f32, tag="mp")
        nc.tensor.matmul(ph_b[:], lhsT=w1sb[:, 128:224], rhs=fx_sb[:], start=True, stop=True)
        hidb = spool.tile([96, SPC], f32)
        nc.scalar.activation(hidb[:], ph_b[:], func=AF.Relu, bias=b1b[:])

        # raw^T = (W2^T @ hid + b2) * vs   [224, SPC]
        pr_a = mpsum.tile([128, SPC], f32, tag="mp")
        nc.tensor.matmul(pr_a[:], lhsT=w2a[:, 0:128], rhs=hida[:], start=True, stop=False)
        nc.tensor.matmul(pr_a[:], lhsT=w2b[:, 0:128], rhs=hidb[:], start=False, stop=True)
        rawa = spool.tile([128, SPC], f32)
        nc.scalar.activation(rawa[:], pr_a[:], func=AF.Identity, bias=b2v_a[:], scale=WSCALE)
        pr_b = mpsum.tile([96, SPC], f32, tag="mp")
        nc.tensor.matmul(pr_b[:], lhsT=w2a[:, 128:224], rhs=hida[:], start=True, stop=False)
        nc.tensor.matmul(pr_b[:], lhsT=w2b[:, 128:224], rhs=hidb[:], start=False, stop=True)
        rawb = spool.tile([96, SPC], f32)
        nc.scalar.activation(rawb[:], pr_b[:], func=AF.Identity, bias=b2v_b[:], scale=WSCALE)

        nc.sync.dma_start(rawT_d.ap()[0:128, :], rawa[:])
        nc.sync.dma_start(rawT_d.ap()[128:224, :], rawb[:])

        # conv1 stationary: block-diag [4*27, 4*8] bf16, partition order
        # (ky, s, ci, kx): lhsT1[ky*36 + s*9 + ci*3 + kx, s*8 + t] = wt[s][t,ci,ky,kx]
        lhsT1 = cpool.tile([SPC * K27, SPC * TMP], bf16)
        nc.vector.memset(lhsT1[:], 0.0)
        for s in range(SPC):
            for ky in range(K):
                for ci in range(CIN):
                    src = AP(
                        tensor=rawT_d,
                        offset=(ci * K * K + ky * K) * SPC + s,
                        ap=[[SPC, K], [K27 * SPC, TMP]],
                    )
                    p0 = ky * 36 + s * 9 + ci * K
                    nc.gpsimd.dma_start(
                        lhsT1[p0:p0 + K, s * TMP:(s + 1) * TMP], src
                    )
        # conv1 bias vector [32, 1]: bias1[s*8+t] = rawT[216+t, s]
        bias1 = cpool.tile([SPC * TMP, 1], f32)
        for s in range(SPC):
            nc.sync.dma_start(
                bias1[s * TMP:(s + 1) * TMP, :], rawT_d.ap()[216:224, s:s + 1]
            )
        # bias rows need scale 0.1, not 0.1/27 -> multiply by 27
        nc.vector.tensor_scalar_mul(bias1[:], bias1[:], float(K27))

        mlp_ctx.close()  # release MLP PSUM banks for conv pools

        # ================= Stage B prep: padded bf16 planes =================
        # cast whole padded X (incl. zero ring + tail) to bf16
        nc.gpsimd.dma_start(padXb.ap(), padX.ap())
        # padY lives in SBUF: planes (s,t) on 32 partitions, PP bf16 each.
        # conv1 epilogue (ACT) writes the interior directly; zero the ring once.
        ypool_res = ctx.enter_context(tc.tile_pool(name="ypres", bufs=1))
        padY_sb = ypool_res.tile([SPC * TMP, PP], bf16)
        nc.vector.memset(padY_sb[:, 0:PW], 0.0)                    # top row
        nc.vector.memset(padY_sb[:, 225 * PW:PP], 0.0)             # bottom row + tail
        lr = padY_sb[:, PW:225 * PW].rearrange("p (r c) -> p r c", c=PW)
        nc.vector.memset(lr[:, :, 0:1], 0.0)                       # left col
        nc.vector.memset(lr[:, :, 225:226], 0.0)                   # right col

        # ================= Stage B: conv pipeline =================
        ic1 = ctx.enter_context(tc.tile_pool(name="ic1", bufs=2))
        ic2 = ctx.enter_context(tc.tile_pool(name="ic2", bufs=4))
        op_ = ctx.enter_context(tc.tile_pool(name="opool", bufs=2))
        ps1 = ctx.enter_context(tc.tile_pool(name="ps1", bufs=2, space="PSUM"))
        ps2 = ctx.enter_context(tc.tile_pool(name="ps2", bufs=2, space="PSUM"))

        def conv1_iter(r):
            r0 = r * RT
            t1 = ic1.tile([SPC * K27, RT, PW], bf16, name=f"t1_{ep_ctr[0]}_{r}", tag="t1")
            # partition (ky, s, ci, kx) <- padXb plane (s,ci), shifted by ky*PW+kx
            for ky in range(K):
                src = AP(
                    tensor=padXb,
                    offset=(r0 + ky) * PW,
                    ap=[[PP, SPC * CIN], [1, K], [1, RT * PW]],
                )
                nc.sync.dma_start(t1[ky * 36:(ky + 1) * 36], src)
            for jp in range(MM_PER_TILE // 2):
                p1 = ps1.tile([SPC * TMP, 1024], f32,
                              name=f"p1_{ep_ctr[0]}_{r}_{jp}", tag="p1")
                nc.tensor.matmul(
                    p1[:, 0:NFREE], lhsT=lhsT1[:],
                    rhs=t1[:, 4 * jp:4 * jp + 2, 0:W], start=True, stop=True,
                )
                nc.tensor.matmul(
                    p1[:, 512:512 + NFREE], lhsT=lhsT1[:],
                    rhs=t1[:, 4 * jp + 2:4 * jp + 4, 0:W], start=True, stop=True,
                )
                # write y rows (r0+4jp .. +3) straight into padY_sb interior
                dst = AP(
                    tensor=padY_sb.tensor,
                    offset=(1 + r0 + jp * 4) * PW + 1,
                    ap=[[PP, SPC * TMP], [2 * PW, 2], [PW, 2], [1, W]],
                )
                pv = AP(
                    tensor=p1.tensor, offset=0,
                    ap=[[1024, SPC * TMP], [512, 2], [W, 2], [1, W]],
                )
                nc.scalar.activation(dst, pv, func=AF.Identity, bias=bias1[:])

        ep_ctr = [0]
        ACT_EVERY = 5  # ACT gets 2 of every 5 conv2 epilogues

        def conv2_pair(s0, r):
            # two samples (s0, s0+1) share one output-store DMA
            r0 = r * RT
            osb = op_.tile([COUT, 2 * RT * W], bf16,
                           name=f"o_{ep_ctr[0]}_{s0}_{r}", tag="o")
            for si in range(2):
                s = s0 + si
                t2 = ic2.tile([K72, RT, PW], bf16,
                              name=f"t2_{ep_ctr[0]}_{s}_{r}", tag="t2")
                # partition (dy, t, dx) <- padY_sb plane (s,t), shifted dy*PW+dx
                for dy in range(K):
                    src = AP(
                        tensor=padY_sb.tensor,
                        offset=s * TMP * PP + (r0 + dy) * PW,
                        ap=[[PP, TMP], [1, K], [1, RT * PW]],
                    )
                    nc.sync.dma_start(t2[dy * 24:(dy + 1) * 24], src)
                for jp in range(MM_PER_TILE // 2):
                    # two matmuls into the two banks of one [64, 1024] psum tile
                    p2 = ps2.tile([COUT, 1024], f32,
                                  name=f"p2_{ep_ctr[0]}_{s}_{r}_{jp}", tag="p2")
                    nc.tensor.matmul(
                        p2[:, 0:NFREE], lhsT=lhsT2[:],
                        rhs=t2[:, 4 * jp:4 * jp + 2, 0:W], start=True, stop=True,
                    )
                    nc.tensor.matmul(
                        p2[:, 512:512 + NFREE], lhsT=lhsT2[:],
                        rhs=t2[:, 4 * jp + 2:4 * jp + 4, 0:W], start=True, stop=True,
                    )
                    pv = p2.rearrange("p (a b) -> p a b", a=2)[:, :, 0:NFREE]
                    obase = si * RT * W + jp * 2 * NFREE
                    oslice = osb[:, obase:obase + 2 * NFREE].rearrange(
                        "p (a b) -> p a b", a=2)
                    if ep_ctr[0] % ACT_EVERY < 2:  # ACT share of conv2 epilogues
                        nc.scalar.activation(oslice, pv, func=AF.Relu,
                                             bias=cnnb_sb[:])
                    else:
                        nc.vector.tensor_scalar(
                            oslice, pv, cnnb_sb[:], 0.0, op0=ALU.add, op1=ALU.max
                        )
                    ep_ctr[0] += 1
            # bf16 -> f32 upcast during store (SWDGE); both samples, one DMA
            dst = AP(
                tensor=out,
                offset=s0 * COUT * H * W + r0 * W,
                ap=[[H * W, COUT], [COUT * H * W, 2], [1, RT * W]],
            )
            nc.gpsimd.dma_start(dst, osb[:].rearrange("p (a b) -> p a b", a=2))

        for _rep in range(repeat):
            conv1_iter(0)
            conv1_iter(1)
            for r in range(NRT):
                if r + 2 < NRT:
                    conv1_iter(r + 2)
                for s0 in (0, 2):
                    conv2_pair(s0, r)

    nc.compile()
    _CACHE[key] = nc
    return nc


def make_in_maps(X, flat_x, W1, b1, W2, b2, cnn_w, cnn_b):
    X = np.asarray(X, np.float32)
    flat_x = np.asarray(flat_x, np.float32)
    W1 = np.asarray(W1, np.float32)
    b1 = np.asarray(b1, np.float32)
    W2 = np.asarray(W2, np.float32)
    b2 = np.asarray(b2, np.float32)
    cnn_w = np.asarray(cnn_w, np.float32)
    cnn_b = np.asarray(cnn_b, np.float32)

    img = np.zeros((B, CIN, PH, PW), np.float32)
    img[:, :, 1:1 + H, 1:1 + W] = X
    Xp = np.zeros((B, CIN, PP), np.float32)
    Xp[:, :, :PLANE] = img.reshape(B, CIN, PLANE)
    fxT_full = np.ascontiguousarray(flat_x.T)                  # [128, 32]
    cnn_wT = np.ascontiguousarray(
        cnn_w.transpose(2, 1, 3, 0).reshape(K72, COUT))        # [72,64] (dy,t,dx,co)

    in_maps = []
    for i in range(NCORES):
        sl = slice(i * SPC, (i + 1) * SPC)
        in_maps.append({
            "padX": np.ascontiguousarray(Xp[sl].reshape(SPC * CIN, PP)),
            "fxT": np.ascontiguousarray(fxT_full[:, sl]),
            "W1": W1, "b1": b1, "W2": W2, "b2": b2,
            "cnn_wT": cnn_wT, "cnn_b": cnn_b,
        })
    return in_maps


def kernel(X, flat_x, W1, b1, W2, b2, cnn_w, cnn_b):
    nc = build_module()
    in_maps = make_in_maps(X, flat_x, W1, b1, W2, b2, cnn_w, cnn_b)
    res = run_bass_kernel_spmd(nc, in_maps, core_ids=list(range(NCORES)))
    outs = [res.results[i]["out"].reshape(SPC, COUT, H, W) for i in range(NCORES)]
    return np.concatenate(outs, axis=0).astype(np.float32)


# revision 14
# speedup vs baseline: 7675.2835x; 90.3404x over previous
"""Trainium2 Bass kernel for nn_MergeMetaCNN (hypernetwork MLP -> grouped conv -> CNN).

Data-parallel over batch: 32 samples -> 8 NeuronCores, 4 samples each.

Per-core pipeline (all math on device):
  1. MLP (fp32 matmuls): hid = relu(W1^T fxT + b1); rawT = W2^T hid + b2,
     scaled per-row by 0.1/27 (filter part) / 0.1 (bias part).
  2. conv1 (grouped 3x3, per-sample dynamic filters) as ONE matmul pass:
     block-diagonal stationary [4*27, 4*8] (bf16), moving operand = im2col
     tile [108, rows, 226] DMA-gathered from zero-padded bf16 X planes
     (each partition = contiguous shifted window of a padded plane).
  3. conv2 (8->64, 3x3) per sample: stationary [72, 64], moving = im2col
     [72, rows, 226] gathered from padded bf16 y planes.
  4. Epilogue relu(x + b) split across ScalarE/VectorE, bf16 staging,
     upcast to fp32 during the store DMA (SWDGE cast).
"""

import numpy as np
from contextlib import ExitStack

import concourse.bass as bass
import concourse.tile as tile
from concourse import bacc, mybir
from concourse.bass_utils import run_bass_kernel_spmd

AP = bass.AP
f32 = mybir.dt.float32
bf16 = mybir.dt.bfloat16
AF = mybir.ActivationFunctionType
ALU = mybir.AluOpType

# Problem constants (hardcoded per contract)
B, CIN, H, W = 32, 3, 224, 224
TMP, K, FLAT, COUT = 8, 3, 128, 64
MLP_OUT = TMP * CIN * K * K + TMP  # 224
META = 0.1
NCORES = 8
SPC = B // NCORES                  # 4 samples per core
PH, PW = H + 2, W + 2              # 226 (zero-pad 1 on each side)
PLANE = PH * PW                    # 51076
PP = PLANE + 4                     # padded plane stride (tail slack for windows)
K27 = CIN * K * K                  # 27
K72 = TMP * K * K                  # 72
RT = 16                            # image rows per row-tile
NRT = H // RT                      # 14 row-tiles
RMM = 2                            # rows per matmul (PSUM bank: 448 fp32 <= 512)
NFREE = RMM * W                    # 448
MM_PER_TILE = RT // RMM            # 4

_CACHE = {}


def build_module(repeat=1, loop_n=None):
    """Build + compile the single-core Bass module (SPMD across 8 cores).

    repeat>1 duplicates the conv pipeline instructions. loop_n wraps the
    pipeline in a hardware For_i loop executing it loop_n times with a
    constant instruction count -- wall-clock slope over loop_n isolates
    device execution time from NEFF load/dispatch overhead."""
    key = ("nc", repeat, loop_n)
    if key in _CACHE:
        return _CACHE[key]
    nc = bacc.Bacc("TRN2", target_bir_lowering=False, debug=False, num_devices=NCORES)

    # ---- DRAM I/O (per-core shapes) ----
    padX = nc.dram_tensor("padX", [SPC * CIN, PP], f32, kind="ExternalInput")
    fxT = nc.dram_tensor("fxT", [FLAT, SPC], f32, kind="ExternalInput")
    W1 = nc.dram_tensor("W1", [FLAT, MLP_OUT], f32, kind="ExternalInput")
    b1 = nc.dram_tensor("b1", [MLP_OUT], f32, kind="ExternalInput")
    W2 = nc.dram_tensor("W2", [MLP_OUT, MLP_OUT], f32, kind="ExternalInput")
    b2 = nc.dram_tensor("b2", [MLP_OUT], f32, kind="ExternalInput")
    cnn_wT = nc.dram_tensor("cnn_wT", [K72, COUT], f32, kind="ExternalInput")
    cnn_b = nc.dram_tensor("cnn_b", [COUT], f32, kind="ExternalInput")
    out = nc.dram_tensor("out", [SPC, COUT, H * W], f32, kind="ExternalOutput")

    padXb = nc.dram_tensor("padXb", [SPC * CIN, PP], bf16)   # bf16 cast of padX
    rawT_d = nc.dram_tensor("rawT_d", [MLP_OUT, SPC], f32)   # MLP out scratch

    with tile.TileContext(nc) as tc, ExitStack() as ctx:
        cpool = ctx.enter_context(tc.tile_pool(name="consts", bufs=1))
        spool = ctx.enter_context(tc.tile_pool(name="stageA", bufs=1))
        mlp_ctx = ExitStack()
        mpsum = mlp_ctx.enter_context(tc.tile_pool(name="mlp_psum", bufs=2, space="PSUM"))

        # ================= Stage A: MLP + weight prep =================
        w1sb = cpool.tile([FLAT, MLP_OUT], f32)
        nc.sync.dma_start(w1sb[:], W1.ap())
        w2a = cpool.tile([128, MLP_OUT], f32)
        nc.sync.dma_start(w2a[:], W2.ap()[0:128, :])
        w2b = cpool.tile([96, MLP_OUT], f32)
        nc.sync.dma_start(w2b[:], W2.ap()[128:224, :])
        fx_sb = cpool.tile([FLAT, SPC], f32)
        nc.sync.dma_start(fx_sb[:], fxT.ap())
        b1a = cpool.tile([128, 1], f32)
        nc.sync.dma_start(b1a[:], b1.ap()[0:128].unsqueeze(1))
        b1b = cpool.tile([96, 1], f32)
        nc.sync.dma_start(b1b[:], b1.ap()[128:224].unsqueeze(1))
        b2a = cpool.tile([128, 1], f32)
        nc.sync.dma_start(b2a[:], b2.ap()[0:128].unsqueeze(1))
        b2b = cpool.tile([96, 1], f32)
        nc.sync.dma_start(b2b[:], b2.ap()[128:224].unsqueeze(1))
        cnnb_sb = cpool.tile([COUT, 1], f32)
        nc.sync.dma_start(cnnb_sb[:], cnn_b.ap().unsqueeze(1))
        lhsT2 = cpool.tile([K72, COUT], bf16)
        nc.gpsimd.dma_start(lhsT2[:], cnn_wT.ap())  # cast f32 -> bf16

        # uniform scale 0.1/27 on all raw rows; bias rows corrected by x27 later
        WSCALE = META / K27
        b2v_a = cpool.tile([128, 1], f32)
        nc.vector.tensor_scalar_mul(b2v_a[:], b2a[:], WSCALE)
        b2v_b = cpool.tile([96, 1], f32)
        nc.vector.tensor_scalar_mul(b2v_b[:], b2b[:], WSCALE)

        # hid^T = relu(W1^T @ fxT + b1)   [224, SPC] in two partition chunks
        ph_a = mpsum.tile([128, SPC], f32, tag="mp")
        nc.tensor.matmul(ph_a[:], lhsT=w1sb[:, 0:128], rhs=fx_sb[:], start=True, stop=True)
        hida = spool.tile([128, SPC], f32)
        nc.scalar.activation(hida[:], ph_a[:], func=AF.Relu, bias=b1a[:])
        ph_b = mpsum.tile([96, SPC], f32, tag="mp")
        nc.tensor.matmul(ph_b[:], lhsT=w1sb[:, 128:224], rhs=fx_sb[:], start=True, stop=True)
        hidb = spool.tile([96, SPC], f32)
        nc.scalar.activation(hidb[:], ph_b[:], func=AF.Relu, bias=b1b[:])

        # raw^T = (W2^T @ hid + b2) * vs   [224, SPC]
        pr_a = mpsum.tile([128, SPC], f32, tag="mp")
        nc.tensor.matmul(pr_a[:], lhsT=w2a[:, 0:128], rhs=hida[:], start=True, stop=False)
        nc.tensor.matmul(pr_a[:], lhsT=w2b[:, 0:128], rhs=hidb[:], start=False, stop=True)
        rawa = spool.tile([128, SPC], f32)
        nc.scalar.activation(rawa[:], pr_a[:], func=AF.Identity, bias=b2v_a[:], scale=WSCALE)
        pr_b = mpsum.tile([96, SPC], f32, tag="mp")
        nc.tensor.matmul(pr_b[:], lhsT=w2a[:, 128:224], rhs=hida[:], start=True, stop=False)
        nc.tensor.matmul(pr_b[:], lhsT=w2b[:, 128:224], rhs=hidb[:], start=False, stop=True)
        rawb = spool.tile([96, SPC], f32)
        nc.scalar.activation(rawb[:], pr_b[:], func=AF.Identity, bias=b2v_b[:], scale=WSCALE)

        nc.sync.dma_start(rawT_d.ap()[0:128, :], rawa[:])
        nc.sync.dma_start(rawT_d.ap()[128:224, :], rawb[:])

        # conv1 stationary: block-diag [4*27, 4*8] bf16, partition order
        # (ky, s, ci, kx): lhsT1[ky*36 + s*9 + ci*3 + kx, s*8 + t] = wt[s][t,ci,ky,kx]
        lhsT1 = cpool.tile([SPC * K27, SPC * TMP], bf16)
        nc.vector.memset(lhsT1[:], 0.0)
        for s in range(SPC):
            for ky in range(K):
                for ci in range(CIN):
                    src = AP(
                        tensor=rawT_d,
                        offset=(ci * K * K + ky * K) * SPC + s,
                        ap=[[SPC, K], [K27 * SPC, TMP]],
                    )
                    p0 = ky * 36 + s * 9 + ci * K
                    nc.gpsimd.dma_start(
                        lhsT1[p0:p0 + K, s * TMP:(s + 1) * TMP], src
                    )
        # conv1 bias vector [32, 1]: bias1[s*8+t] = rawT[216+t, s]
        bias1 = cpool.tile([SPC * TMP, 1], f32)
        for s in range(SPC):
            nc.sync.dma_start(
                bias1[s * TMP:(s + 1) * TMP, :], rawT_d.ap()[216:224, s:s + 1]
            )
        # bias rows need scale 0.1, not 0.1/27 -> multiply by 27
        nc.vector.tensor_scalar_mul(bias1[:], bias1[:], float(K27))

        mlp_ctx.close()  # release MLP PSUM banks for conv pools

        # ================= Stage B prep: padded bf16 planes =================
        # cast whole padded X (incl. zero ring + tail) to bf16
        nc.gpsimd.dma_start(padXb.ap(), padX.ap())
        # padY lives in SBUF: planes (s,t) on 32 partitions, PP bf16 each.
        # conv1 epilogue (ACT) writes the interior directly; zero the ring once.
        ypool_res = ctx.enter_context(tc.tile_pool(name="ypres", bufs=1))
        padY_sb = ypool_res.tile([SPC * TMP, PP], bf16)
        nc.vector.memset(padY_sb[:, 0:PW], 0.0)                    # top row
        nc.vector.memset(padY_sb[:, 225 * PW:PP], 0.0)             # bottom row + tail
        lr = padY_sb[:, PW:225 * PW].rearrange("p (r c) -> p r c", c=PW)
        nc.vector.memset(lr[:, :, 0:1], 0.0)                       # left col
        nc.vector.memset(lr[:, :, 225:226], 0.0)                   # right col

        # ================= Stage B: conv pipeline =================
        ic1 = ctx.enter_context(tc.tile_pool(name="ic1", bufs=2))
        ic2 = ctx.enter_context(tc.tile_pool(name="ic2", bufs=4))
        op_ = ctx.enter_context(tc.tile_pool(name="opool", bufs=2))
        ps1 = ctx.enter_context(tc.tile_pool(name="ps1", bufs=2, space="PSUM"))
        ps2 = ctx.enter_context(tc.tile_pool(name="ps2", bufs=2, space="PSUM"))

        def conv1_iter(r):
            r0 = r * RT
            t1 = ic1.tile([SPC * K27, RT, PW], bf16, name=f"t1_{ep_ctr[0]}_{r}", tag="t1")
            # partition (ky, s, ci, kx) <- padXb plane (s,ci), shifted by ky*PW+kx
            for ky in range(K):
                src = AP(
                    tensor=padXb,
                    offset=(r0 + ky) * PW,
                    ap=[[PP, SPC * CIN], [1, K], [1, RT * PW]],
                )
                nc.sync.dma_start(t1[ky * 36:(ky + 1) * 36], src)
            for jp in range(MM_PER_TILE // 2):
                p1 = ps1.tile([SPC * TMP, 1024], f32,
                              name=f"p1_{ep_ctr[0]}_{r}_{jp}", tag="p1")
                nc.tensor.matmul(
                    p1[:, 0:NFREE], lhsT=lhsT1[:],
                    rhs=t1[:, 4 * jp:4 * jp + 2, 0:W], start=True, stop=True,
                )
                nc.tensor.matmul(
                    p1[:, 512:512 + NFREE], lhsT=lhsT1[:],
                    rhs=t1[:, 4 * jp + 2:4 * jp + 4, 0:W], start=True, stop=True,
                )
                # write y rows (r0+4jp .. +3) straight into padY_sb interior
                dst = AP(
                    tensor=padY_sb.tensor,
                    offset=(1 + r0 + jp * 4) * PW + 1,
                    ap=[[PP, SPC * TMP], [2 * PW, 2], [PW, 2], [1, W]],
                )
                pv = AP(
                    tensor=p1.tensor, offset=0,
                    ap=[[1024, SPC * TMP], [512, 2], [W, 2], [1, W]],
                )
                nc.scalar.activation(dst, pv, func=AF.Identity, bias=bias1[:])

        ep_ctr = [0]
        ACT_EVERY = 5  # ACT gets 2 of every 5 conv2 epilogues

        def conv2_pair(s0, r):
            # two samples (s0, s0+1) share one output-store DMA
            r0 = r * RT
            osb = op_.tile([COUT, 2 * RT * W], bf16,
                           name=f"o_{ep_ctr[0]}_{s0}_{r}", tag="o")
            for si in range(2):
                s = s0 + si
                t2 = ic2.tile([K72, RT, PW], bf16,
                              name=f"t2_{ep_ctr[0]}_{s}_{r}", tag="t2")
                # partition (dy, t, dx) <- padY_sb plane (s,t), shifted dy*PW+dx
                for dy in range(K):
                    src = AP(
                        tensor=padY_sb.tensor,
                        offset=s * TMP * PP + (r0 + dy) * PW,
                        ap=[[PP, TMP], [1, K], [1, RT * PW]],
                    )
                    nc.sync.dma_start(t2[dy * 24:(dy + 1) * 24], src)
                for jp in range(MM_PER_TILE // 2):
                    # two matmuls into the two banks of one [64, 1024] psum tile
                    p2 = ps2.tile([COUT, 1024], f32,
                                  name=f"p2_{ep_ctr[0]}_{s}_{r}_{jp}", tag="p2")
                    nc.tensor.matmul(
                        p2[:, 0:NFREE], lhsT=lhsT2[:],
                        rhs=t2[:, 4 * jp:4 * jp + 2, 0:W], start=True, stop=True,
                    )
                    nc.tensor.matmul(
                        p2[:, 512:512 + NFREE], lhsT=lhsT2[:],
                        rhs=t2[:, 4 * jp + 2:4 * jp + 4, 0:W], start=True, stop=True,
                    )
                    pv = p2.rearrange("p (a b) -> p a b", a=2)[:, :, 0:NFREE]
                    obase = si * RT * W + jp * 2 * NFREE
                    oslice = osb[:, obase:obase + 2 * NFREE].rearrange(
                        "p (a b) -> p a b", a=2)
                    if ep_ctr[0] % ACT_EVERY < 2:  # ACT share of conv2 epilogues
                        nc.scalar.activation(oslice, pv, func=AF.Relu,
                                             bias=cnnb_sb[:])
                    else:
                        nc.vector.tensor_scalar(
                            oslice, pv, cnnb_sb[:], 0.0, op0=ALU.add, op1=ALU.max
                        )
                    ep_ctr[0] += 1
            # bf16 -> f32 upcast during store (SWDGE); both samples, one DMA
            dst = AP(
                tensor=out,
                offset=s0 * COUT * H * W + r0 * W,
                ap=[[H * W, COUT], [COUT * H * W, 2], [1, RT * W]],
            )
            nc.gpsimd.dma_start(dst, osb[:].rearrange("p (a b) -> p a b", a=2))

        def pipeline():
            conv1_iter(0)
            conv1_iter(1)
            for r in range(NRT):
                if r + 2 < NRT:
                    conv1_iter(r + 2)
                for s0 in (0, 2):
                    conv2_pair(s0, r)

        if loop_n is not None:
            with tc.For_i(0, loop_n, 1):
                pipeline()
        else:
            for _rep in range(repeat):
                pipeline()

    nc.compile()
    _CACHE[key] = nc
    return nc


def make_in_maps(X, flat_x, W1, b1, W2, b2, cnn_w, cnn_b):
    X = np.asarray(X, np.float32)
    flat_x = np.asarray(flat_x, np.float32)
    W1 = np.asarray(W1, np.float32)
    b1 = np.asarray(b1, np.float32)
    W2 = np.asarray(W2, np.float32)
    b2 = np.asarray(b2, np.float32)
    cnn_w = np.asarray(cnn_w, np.float32)
    cnn_b = np.asarray(cnn_b, np.float32)

    img = np.zeros((B, CIN, PH, PW), np.float32)
    img[:, :, 1:1 + H, 1:1 + W] = X
    Xp = np.zeros((B, CIN, PP), np.float32)
    Xp[:, :, :PLANE] = img.reshape(B, CIN, PLANE)
    fxT_full = np.ascontiguousarray(flat_x.T)                  # [128, 32]
    cnn_wT = np.ascontiguousarray(
        cnn_w.transpose(2, 1, 3, 0).reshape(K72, COUT))        # [72,64] (dy,t,dx,co)

    in_maps = []
    for i in range(NCORES):
        sl = slice(i * SPC, (i + 1) * SPC)
        in_maps.append({
            "padX": np.ascontiguousarray(Xp[sl].reshape(SPC * CIN, PP)),
            "fxT": np.ascontiguousarray(fxT_full[:, sl]),
            "W1": W1, "b1": b1, "W2": W2, "b2": b2,
            "cnn_wT": cnn_wT, "cnn_b": cnn_b,
        })
    return in_maps


def kernel(X, flat_x, W1, b1, W2, b2, cnn_w, cnn_b):
    nc = build_module()
    in_maps = make_in_maps(X, flat_x, W1, b1, W2, b2, cnn_w, cnn_b)
    res = run_bass_kernel_spmd(nc, in_maps, core_ids=list(range(NCORES)))
    outs = [res.results[i]["out"].reshape(SPC, COUT, H, W) for i in range(NCORES)]
    return np.concatenate(outs, axis=0).astype(np.float32)


# revision 15
# speedup vs baseline: 9120.4297x; 1.1883x over previous
"""Trainium2 Bass kernel for nn_MergeMetaCNN (hypernetwork MLP -> grouped conv -> CNN).

Data-parallel over batch: 32 samples -> 8 NeuronCores, 4 samples each.

Per-core pipeline (all math on device):
  1. MLP (fp32 matmuls): hid = relu(W1^T fxT + b1); rawT = W2^T hid + b2,
     scaled per-row by 0.1/27 (filter part) / 0.1 (bias part).
  2. conv1 (grouped 3x3, per-sample dynamic filters) as ONE matmul pass:
     block-diagonal stationary [4*27, 4*8] (bf16), moving operand = im2col
     tile [108, rows, 226] DMA-gathered from zero-padded bf16 X planes
     (each partition = contiguous shifted window of a padded plane).
  3. conv2 (8->64, 3x3) per sample: stationary [72, 64], moving = im2col
     [72, rows, 226] gathered from padded bf16 y planes.
  4. Epilogue relu(x + b) split across ScalarE/VectorE, bf16 staging,
     upcast to fp32 during the store DMA (SWDGE cast).
"""

import numpy as np
from contextlib import ExitStack

import concourse.bass as bass
import concourse.tile as tile
from concourse import bacc, mybir
from concourse.bass_utils import run_bass_kernel_spmd

AP = bass.AP
f32 = mybir.dt.float32
bf16 = mybir.dt.bfloat16
AF = mybir.ActivationFunctionType
ALU = mybir.AluOpType

# Problem constants (hardcoded per contract)
B, CIN, H, W = 32, 3, 224, 224
TMP, K, FLAT, COUT = 8, 3, 128, 64
MLP_OUT = TMP * CIN * K * K + TMP  # 224
META = 0.1
NCORES = 8
SPC = B // NCORES                  # 4 samples per core
PH, PW = H + 2, W + 2              # 226 (zero-pad 1 on each side)
PLANE = PH * PW                    # 51076
PP = PLANE + 4                     # padded plane stride (tail slack for windows)
K27 = CIN * K * K                  # 27
K72 = TMP * K * K                  # 72
RT = 16                            # image rows per row-tile
NRT = H // RT                      # 14 row-tiles
RMM = 2                            # rows per matmul (PSUM bank: 448 fp32 <= 512)
NFREE = RMM * W                    # 448
MM_PER_TILE = RT // RMM            # 4

_CACHE = {}


def build_module(repeat=1, loop_n=None):
    """Build + compile the single-core Bass module (SPMD across 8 cores).

    repeat>1 duplicates the conv pipeline instructions. loop_n wraps the
    pipeline in a hardware For_i loop executing it loop_n times with a
    constant instruction count -- wall-clock slope over loop_n isolates
    device execution time from NEFF load/dispatch overhead."""
    key = ("nc", repeat, loop_n)
    if key in _CACHE:
        return _CACHE[key]
    nc = bacc.Bacc("TRN2", target_bir_lowering=False, debug=False, num_devices=NCORES)

    # ---- DRAM I/O (per-core shapes) ----
    padX = nc.dram_tensor("padX", [SPC * CIN, PP], f32, kind="ExternalInput")
    fxT = nc.dram_tensor("fxT", [FLAT, SPC], f32, kind="ExternalInput")
    W1 = nc.dram_tensor("W1", [FLAT, MLP_OUT], f32, kind="ExternalInput")
    b1 = nc.dram_tensor("b1", [MLP_OUT], f32, kind="ExternalInput")
    W2 = nc.dram_tensor("W2", [MLP_OUT, MLP_OUT], f32, kind="ExternalInput")
    b2 = nc.dram_tensor("b2", [MLP_OUT], f32, kind="ExternalInput")
    cnn_wT = nc.dram_tensor("cnn_wT", [K72, COUT], f32, kind="ExternalInput")
    cnn_b = nc.dram_tensor("cnn_b", [COUT], f32, kind="ExternalInput")
    out = nc.dram_tensor("out", [SPC, COUT, H * W], f32, kind="ExternalOutput")

    padXb = nc.dram_tensor("padXb", [SPC * CIN, PP], bf16)   # bf16 cast of padX
    rawT_d = nc.dram_tensor("rawT_d", [MLP_OUT, SPC], f32)   # MLP out scratch

    with tile.TileContext(nc) as tc, ExitStack() as ctx:
        cpool = ctx.enter_context(tc.tile_pool(name="consts", bufs=1))
        spool = ctx.enter_context(tc.tile_pool(name="stageA", bufs=1))
        mlp_ctx = ExitStack()
        mpsum = mlp_ctx.enter_context(tc.tile_pool(name="mlp_psum", bufs=2, space="PSUM"))

        # ================= Stage A: MLP + weight prep =================
        w1sb = cpool.tile([FLAT, MLP_OUT], f32)
        nc.sync.dma_start(w1sb[:], W1.ap())
        w2a = cpool.tile([128, MLP_OUT], f32)
        nc.sync.dma_start(w2a[:], W2.ap()[0:128, :])
        w2b = cpool.tile([96, MLP_OUT], f32)
        nc.sync.dma_start(w2b[:], W2.ap()[128:224, :])
        fx_sb = cpool.tile([FLAT, SPC], f32)
        nc.sync.dma_start(fx_sb[:], fxT.ap())
        b1a = cpool.tile([128, 1], f32)
        nc.sync.dma_start(b1a[:], b1.ap()[0:128].unsqueeze(1))
        b1b = cpool.tile([96, 1], f32)
        nc.sync.dma_start(b1b[:], b1.ap()[128:224].unsqueeze(1))
        b2a = cpool.tile([128, 1], f32)
        nc.sync.dma_start(b2a[:], b2.ap()[0:128].unsqueeze(1))
        b2b = cpool.tile([96, 1], f32)
        nc.sync.dma_start(b2b[:], b2.ap()[128:224].unsqueeze(1))
        cnnb_sb = cpool.tile([COUT, 1], f32)
        nc.sync.dma_start(cnnb_sb[:], cnn_b.ap().unsqueeze(1))
        lhsT2 = cpool.tile([K72, COUT], bf16)
        nc.gpsimd.dma_start(lhsT2[:], cnn_wT.ap())  # cast f32 -> bf16

        # uniform scale 0.1/27 on all raw rows; bias rows corrected by x27 later
        WSCALE = META / K27
        b2v_a = cpool.tile([128, 1], f32)
        nc.vector.tensor_scalar_mul(b2v_a[:], b2a[:], WSCALE)
        b2v_b = cpool.tile([96, 1], f32)
        nc.vector.tensor_scalar_mul(b2v_b[:], b2b[:], WSCALE)

        # hid^T = relu(W1^T @ fxT + b1)   [224, SPC] in two partition chunks
        ph_a = mpsum.tile([128, SPC], f32, tag="mp")
        nc.tensor.matmul(ph_a[:], lhsT=w1sb[:, 0:128], rhs=fx_sb[:], start=True, stop=True)
        hida = spool.tile([128, SPC], f32)
        nc.scalar.activation(hida[:], ph_a[:], func=AF.Relu, bias=b1a[:])
        ph_b = mpsum.tile([96, SPC], f32, tag="mp")
        nc.tensor.matmul(ph_b[:], lhsT=w1sb[:, 128:224], rhs=fx_sb[:], start=True, stop=True)
        hidb = spool.tile([96, SPC], f32)
        nc.scalar.activation(hidb[:], ph_b[:], func=AF.Relu, bias=b1b[:])

        # raw^T = (W2^T @ hid + b2) * vs   [224, SPC]
        pr_a = mpsum.tile([128, SPC], f32, tag="mp")
        nc.tensor.matmul(pr_a[:], lhsT=w2a[:, 0:128], rhs=hida[:], start=True, stop=False)
        nc.tensor.matmul(pr_a[:], lhsT=w2b[:, 0:128], rhs=hidb[:], start=False, stop=True)
        rawa = spool.tile([128, SPC], f32)
        nc.scalar.activation(rawa[:], pr_a[:], func=AF.Identity, bias=b2v_a[:], scale=WSCALE)
        pr_b = mpsum.tile([96, SPC], f32, tag="mp")
        nc.tensor.matmul(pr_b[:], lhsT=w2a[:, 128:224], rhs=hida[:], start=True, stop=False)
        nc.tensor.matmul(pr_b[:], lhsT=w2b[:, 128:224], rhs=hidb[:], start=False, stop=True)
        rawb = spool.tile([96, SPC], f32)
        nc.scalar.activation(rawb[:], pr_b[:], func=AF.Identity, bias=b2v_b[:], scale=WSCALE)

        nc.sync.dma_start(rawT_d.ap()[0:128, :], rawa[:])
        nc.sync.dma_start(rawT_d.ap()[128:224, :], rawb[:])

        # conv1 stationary: block-diag [4*27, 4*8] bf16, partition order
        # (ky, s, ci, kx): lhsT1[ky*36 + s*9 + ci*3 + kx, s*8 + t] = wt[s][t,ci,ky,kx]
        lhsT1 = cpool.tile([SPC * K27, SPC * TMP], bf16)
        nc.vector.memset(lhsT1[:], 0.0)
        for s in range(SPC):
            for ky in range(K):
                for ci in range(CIN):
                    src = AP(
                        tensor=rawT_d,
                        offset=(ci * K * K + ky * K) * SPC + s,
                        ap=[[SPC, K], [K27 * SPC, TMP]],
                    )
                    p0 = ky * 36 + s * 9 + ci * K
                    nc.gpsimd.dma_start(
                        lhsT1[p0:p0 + K, s * TMP:(s + 1) * TMP], src
                    )
        # conv1 bias vector [32, 1]: bias1[s*8+t] = rawT[216+t, s]
        bias1 = cpool.tile([SPC * TMP, 1], f32)
        for s in range(SPC):
            nc.sync.dma_start(
                bias1[s * TMP:(s + 1) * TMP, :], rawT_d.ap()[216:224, s:s + 1]
            )
        # bias rows need scale 0.1, not 0.1/27 -> multiply by 27
        nc.vector.tensor_scalar_mul(bias1[:], bias1[:], float(K27))

        mlp_ctx.close()  # release MLP PSUM banks for conv pools

        # ================= Stage B prep: padded bf16 planes =================
        # cast whole padded X (incl. zero ring + tail) to bf16
        nc.gpsimd.dma_start(padXb.ap(), padX.ap())
        # padY lives in SBUF: planes (s,t) on 32 partitions, PP bf16 each.
        # conv1 epilogue (ACT) writes the interior directly; zero the ring once.
        ypool_res = ctx.enter_context(tc.tile_pool(name="ypres", bufs=1))
        padY_sb = ypool_res.tile([SPC * TMP, PP], bf16)
        nc.vector.memset(padY_sb[:, 0:PW], 0.0)                    # top row
        nc.vector.memset(padY_sb[:, 225 * PW:PP], 0.0)             # bottom row + tail
        lr = padY_sb[:, PW:225 * PW].rearrange("p (r c) -> p r c", c=PW)
        nc.vector.memset(lr[:, :, 0:1], 0.0)                       # left col
        nc.vector.memset(lr[:, :, 225:226], 0.0)                   # right col

        # ================= Stage B: conv pipeline =================
        ic1 = ctx.enter_context(tc.tile_pool(name="ic1", bufs=2))
        ic2 = ctx.enter_context(tc.tile_pool(name="ic2", bufs=4))
        op_ = ctx.enter_context(tc.tile_pool(name="opool", bufs=2))
        ps1 = ctx.enter_context(tc.tile_pool(name="ps1", bufs=2, space="PSUM"))
        ps2 = ctx.enter_context(tc.tile_pool(name="ps2", bufs=2, space="PSUM"))

        def conv1_iter(r):
            r0 = r * RT
            t1 = ic1.tile([SPC * K27, RT, PW], bf16, name=f"t1_{ep_ctr[0]}_{r}", tag="t1")
            # partition (ky, s, ci, kx) <- padXb plane (s,ci), shifted by ky*PW+kx
            for ky in range(K):
                src = AP(
                    tensor=padXb,
                    offset=(r0 + ky) * PW,
                    ap=[[PP, SPC * CIN], [1, K], [1, RT * PW]],
                )
                nc.sync.dma_start(t1[ky * 36:(ky + 1) * 36], src)
            for jp in range(MM_PER_TILE // 2):
                p1 = ps1.tile([SPC * TMP, 1024], f32,
                              name=f"p1_{ep_ctr[0]}_{r}_{jp}", tag="p1")
                nc.tensor.matmul(
                    p1[:, 0:NFREE], lhsT=lhsT1[:],
                    rhs=t1[:, 4 * jp:4 * jp + 2, 0:W], start=True, stop=True,
                )
                nc.tensor.matmul(
                    p1[:, 512:512 + NFREE], lhsT=lhsT1[:],
                    rhs=t1[:, 4 * jp + 2:4 * jp + 4, 0:W], start=True, stop=True,
                )
                # write y rows (r0+4jp .. +3) straight into padY_sb interior
                dst = AP(
                    tensor=padY_sb.tensor,
                    offset=(1 + r0 + jp * 4) * PW + 1,
                    ap=[[PP, SPC * TMP], [2 * PW, 2], [PW, 2], [1, W]],
                )
                pv = AP(
                    tensor=p1.tensor, offset=0,
                    ap=[[1024, SPC * TMP], [512, 2], [W, 2], [1, W]],
                )
                nc.scalar.activation(dst, pv, func=AF.Identity, bias=bias1[:])

        ep_ctr = [0]
        ACT_EVERY = 5  # ACT gets 2 of every 5 conv2 epilogues

        def conv2_pair(s0, r):
            # two samples (s0, s0+1) share one output-store DMA
            r0 = r * RT
            osb = op_.tile([COUT, 2 * RT * W], bf16,
                           name=f"o_{ep_ctr[0]}_{s0}_{r}", tag="o")
            for si in range(2):
                s = s0 + si
                t2 = ic2.tile([K72, RT, PW], bf16,
                              name=f"t2_{ep_ctr[0]}_{s}_{r}", tag="t2")
                # partition (dy, t, dx) <- padY_sb plane (s,t), shifted dy*PW+dx
                for dy in range(K):
                    src = AP(
                        tensor=padY_sb.tensor,
                        offset=s * TMP * PP + (r0 + dy) * PW,
                        ap=[[PP, TMP], [1, K], [1, RT * PW]],
                    )
                    nc.sync.dma_start(t2[dy * 24:(dy + 1) * 24], src)
                for jp in range(MM_PER_TILE // 2):
                    # two matmuls into the two banks of one [64, 1024] psum tile
                    p2 = ps2.tile([COUT, 1024], f32,
                                  name=f"p2_{ep_ctr[0]}_{s}_{r}_{jp}", tag="p2")
                    nc.tensor.matmul(
                        p2[:, 0:NFREE], lhsT=lhsT2[:],
                        rhs=t2[:, 4 * jp:4 * jp + 2, 0:W], start=True, stop=True,
                    )
                    nc.tensor.matmul(
                        p2[:, 512:512 + NFREE], lhsT=lhsT2[:],
                        rhs=t2[:, 4 * jp + 2:4 * jp + 4, 0:W], start=True, stop=True,
                    )
                    pv = p2.rearrange("p (a b) -> p a b", a=2)[:, :, 0:NFREE]
                    obase = si * RT * W + jp * 2 * NFREE
                    oslice = osb[:, obase:obase + 2 * NFREE].rearrange(
                        "p (a b) -> p a b", a=2)
                    if ep_ctr[0] % ACT_EVERY < 2:  # ACT share of conv2 epilogues
                        nc.scalar.activation(oslice, pv, func=AF.Relu,
                                             bias=cnnb_sb[:])
                    else:
                        nc.vector.tensor_scalar(
                            oslice, pv, cnnb_sb[:], 0.0, op0=ALU.add, op1=ALU.max
                        )
                    ep_ctr[0] += 1
            # bf16 -> f32 upcast during store (SWDGE); both samples, one DMA
            dst = AP(
                tensor=out,
                offset=s0 * COUT * H * W + r0 * W,
                ap=[[H * W, COUT], [COUT * H * W, 2], [1, RT * W]],
            )
            nc.gpsimd.dma_start(dst, osb[:].rearrange("p (a b) -> p a b", a=2))

        def pipeline():
            conv1_iter(0)
            conv1_iter(1)
            for r in range(NRT):
                if r + 2 < NRT:
                    conv1_iter(r + 2)
                for s0 in (0, 2):
                    conv2_pair(s0, r)

        if loop_n is not None:
            hints = [mybir.EngineType.PE, mybir.EngineType.Activation,
                     mybir.EngineType.DVE, mybir.EngineType.SP,
                     mybir.EngineType.Pool]
            with tc.For_i(0, loop_n, 1, hint_engines=hints):
                pipeline()
        else:
            for _rep in range(repeat):
                pipeline()

    nc.compile()
    _CACHE[key] = nc
    return nc


def make_in_maps(X, flat_x, W1, b1, W2, b2, cnn_w, cnn_b):
    X = np.asarray(X, np.float32)
    flat_x = np.asarray(flat_x, np.float32)
    W1 = np.asarray(W1, np.float32)
    b1 = np.asarray(b1, np.float32)
    W2 = np.asarray(W2, np.float32)
    b2 = np.asarray(b2, np.float32)
    cnn_w = np.asarray(cnn_w, np.float32)
    cnn_b = np.asarray(cnn_b, np.float32)

    img = np.zeros((B, CIN, PH, PW), np.float32)
    img[:, :, 1:1 + H, 1:1 + W] = X
    Xp = np.zeros((B, CIN, PP), np.float32)
    Xp[:, :, :PLANE] = img.reshape(B, CIN, PLANE)
    fxT_full = np.ascontiguousarray(flat_x.T)                  # [128, 32]
    cnn_wT = np.ascontiguousarray(
        cnn_w.transpose(2, 1, 3, 0).reshape(K72, COUT))        # [72,64] (dy,t,dx,co)

    in_maps = []
    for i in range(NCORES):
        sl = slice(i * SPC, (i + 1) * SPC)
        in_maps.append({
            "padX": np.ascontiguousarray(Xp[sl].reshape(SPC * CIN, PP)),
            "fxT": np.ascontiguousarray(fxT_full[:, sl]),
            "W1": W1, "b1": b1, "W2": W2, "b2": b2,
            "cnn_wT": cnn_wT, "cnn_b": cnn_b,
        })
    return in_maps


def kernel(X, flat_x, W1, b1, W2, b2, cnn_w, cnn_b):
    nc = build_module()
    in_maps = make_in_maps(X, flat_x, W1, b1, W2, b2, cnn_w, cnn_b)
    res = run_bass_kernel_spmd(nc, in_maps, core_ids=list(range(NCORES)))
    outs = [res.results[i]["out"].reshape(SPC, COUT, H, W) for i in range(NCORES)]
    return np.concatenate(outs, axis=0).astype(np.float32)
